# revision 9
# baseline (speedup 1.0000x reference)
"""Trainium2 Bass kernel for nn_BertEncoder_45260365365543.

BERT-base encoder (12 layers, B=4, S=512, H=768) + pairwise L2-distance
outputs, on 8 NeuronCores:
  - 4 pairs of cores; pair p handles batch element p.
  - Within a pair: tensor-parallel split (6 of 12 heads, half of the FFN
    per core) with two pairwise AllReduces per layer (after the Wo partial
    and after the W2 partial).
  - Final pairwise-distance block: both cores of a pair hold the full
    embedded sequence after the last AllReduce; each computes its half of
    the query rows of word_word / word_operator locally. One global
    AllReduce produces the distance means.

All compute is fp32 (PE fp32 matmuls, fp32 PSUM accumulation) to keep the
sigmoid-threshold outputs numerically faithful to the fp32 reference.
"""

import os
import sys

sys.path.insert(0, "/opt/trn_rl_repo")

import numpy as np

import concourse.bass as bass
import concourse.mybir as mybir
import concourse.tile as tile
from concourse import bacc, bass2jax
from concourse.masks import make_identity

import jax
from jax.sharding import Mesh, PartitionSpec, NamedSharding
from jax.experimental.shard_map import shard_map

F32 = mybir.dt.float32
AF = mybir.ActivationFunctionType
ALU = mybir.AluOpType

NL = int(os.environ.get("BK_NL", "12"))
USE_R = os.environ.get("BK_R", "0") == "1"
NO_AR = os.environ.get("BK_NOAR", "0") == "1"   # timing ablation only
F32R = mybir.dt.float32r
H, NH, DH, FF, S, B = 768, 12, 64, 3072, 512, 4
NOPS = 16
THRESH = 0.4
EPS = 1e-12
SCALE = 1.0 / float(np.sqrt(DH))

HHALF = H // 2            # 384 features (6 heads) per core
FFHALF = FF // 2          # 1536
NSC = S // 128            # 4 sequence chunks
NHC = H // 128            # 6 feature chunks
NFFC = FFHALF // 128      # 12 ff chunks per core
NQK = HHALF // 128        # 3 chunks for Q^T / K^T halves
NHEADS = NH // 2          # 6 heads per core
SQ = S // 2               # query rows handled per core

N_CORES = 8
PAIRS = [[2 * p, 2 * p + 1] for p in range(4)]


def build_program():
    nc = bacc.Bacc()

    # ---------------- DRAM I/O ----------------
    x0_d = nc.dram_tensor("x0", [S, H], F32, kind="ExternalInput")
    amask_d = nc.dram_tensor("amask", [128, NSC], F32, kind="ExternalInput")
    embg_d = nc.dram_tensor("embg", [128, H], F32, kind="ExternalInput")
    embb_d = nc.dram_tensor("embb", [128, H], F32, kind="ExternalInput")
    wqkv_d = nc.dram_tensor("wqkv", [NL, H, 3 * HHALF], F32, kind="ExternalInput")
    wo_d = nc.dram_tensor("wo", [NL, HHALF, H], F32, kind="ExternalInput")
    w1_d = nc.dram_tensor("w1", [NL, H, FFHALF], F32, kind="ExternalInput")
    w2_d = nc.dram_tensor("w2", [NL, FFHALF, H], F32, kind="ExternalInput")
    qkb_d = nc.dram_tensor("qkb", [NL, 128, 2 * NQK + NFFC], F32,
                           kind="ExternalInput")
    vb_d = nc.dram_tensor("vb", [NL, 128, HHALF], F32, kind="ExternalInput")
    ob_d = nc.dram_tensor("ob", [NL, 128, H], F32, kind="ExternalInput")
    b2_d = nc.dram_tensor("b2", [NL, 128, H], F32, kind="ExternalInput")
    ln1g_d = nc.dram_tensor("ln1g", [NL, 128, H], F32, kind="ExternalInput")
    ln1b_d = nc.dram_tensor("ln1b", [NL, 128, H], F32, kind="ExternalInput")
    ln2g_d = nc.dram_tensor("ln2g", [NL, 128, H], F32, kind="ExternalInput")
    ln2b_d = nc.dram_tensor("ln2b", [NL, 128, H], F32, kind="ExternalInput")
    opT_d = nc.dram_tensor("opT", [H, NOPS], F32, kind="ExternalInput")
    scal_d = nc.dram_tensor("scal", [1, 2], F32, kind="ExternalInput")
    hsel_d = nc.dram_tensor("hsel", [128, 2], F32, kind="ExternalInput")

    emb_out = nc.dram_tensor("emb_out", [SQ, H], F32, kind="ExternalOutput")
    ww_out = nc.dram_tensor("ww_out", [SQ, S], F32, kind="ExternalOutput")
    wop_out = nc.dram_tensor("wop_out", [SQ, NOPS], F32, kind="ExternalOutput")

    from contextlib import ExitStack
    with tile.TileContext(nc) as tc, ExitStack() as ctx:
        const = ctx.enter_context(tc.tile_pool(name="const", bufs=1))
        sb = ctx.enter_context(tc.tile_pool(name="sb", bufs=1))
        stat = ctx.enter_context(tc.tile_pool(name="stat", bufs=12))
        wpool = ctx.enter_context(tc.tile_pool(name="wpool", bufs=4))
        ps = ctx.enter_context(tc.tile_pool(name="ps", bufs=4, space="PSUM"))
        ps2 = ctx.enter_context(tc.tile_pool(name="ps2", bufs=2, space="PSUM"))
        dram = ctx.enter_context(tc.tile_pool(name="dram", bufs=1, space="DRAM"))

        ones = const.tile([128, 128], F32)
        nc.gpsimd.memset(ones[:], 1.0)
        ident = const.tile([128, 128], F32)
        make_identity(nc, ident[:])
        amask = const.tile([128, NSC], F32)
        nc.sync.dma_start(amask[:], amask_d[:])
        scal = const.tile([1, 2], F32)
        nc.sync.dma_start(scal[:], scal_d[:])
        hsel = const.tile([128, 2], F32)
        nc.sync.dma_start(hsel[:], hsel_d[:])
        epst = const.tile([128, 1], F32)
        nc.gpsimd.memset(epst[:], EPS)

        def transpose_128(out_slice, in_slice):
            tp = ps2.tile([128, 128], F32, tag="tp", name="tp")
            nc.tensor.transpose(tp[:], in_slice, ident[:])
            nc.vector.tensor_copy(out_slice, tp[:])

        def ln_natural(x_slice, g_tile, b_tile, out_slice):
            """LayerNorm along the free dim of a [128, H] slice."""
            mus = stat.tile([128, 1], F32, tag="stat", name="mus")
            nc.vector.tensor_reduce(mus[:], x_slice, axis=mybir.AxisListType.X,
                                    op=ALU.add)
            mu = stat.tile([128, 1], F32, tag="stat", name="mu")
            nc.scalar.mul(mu[:], mus[:], 1.0 / H)
            xc = sb.tile([128, H], F32, tag="lns", bufs=4, name="xc")
            nc.vector.tensor_scalar_sub(xc[:], x_slice, mu[:])
            sq = sb.tile([128, H], F32, tag="lns", bufs=4, name="sq")
            ss = stat.tile([128, 1], F32, tag="stat", name="ss")
            nc.scalar.activation(sq[:], xc[:], AF.Square, accum_out=ss[:])
            std = stat.tile([128, 1], F32, tag="stat", name="std")
            nc.scalar.activation(std[:], ss[:], AF.Sqrt, bias=epst[:], scale=1.0 / H)
            istd = stat.tile([128, 1], F32, tag="stat", name="istd")
            nc.vector.reciprocal(istd[:], std[:])
            t2 = sb.tile([128, H], F32, tag="lns", bufs=4, name="t2")
            nc.vector.scalar_tensor_tensor(t2[:], xc[:], istd[:], g_tile,
                                           op0=ALU.mult, op1=ALU.mult)
            nc.vector.tensor_add(out_slice, t2[:], b_tile)

        def load_matrix(dram3, li, nrows, ncols, slabs_per_tile):
            """Load [nrows, ncols] matrix (layer li) into tiles of
            slabs_per_tile row-slabs each; returns slab accessor."""
            nslab = nrows // 128
            tiles = []
            for t0 in range((nslab + slabs_per_tile - 1) // slabs_per_tile):
                cnt = min(slabs_per_tile, nslab - t0 * slabs_per_tile)
                wdt = F32R if USE_R else F32
                wt = wpool.tile([128, cnt * ncols], wdt, tag="wb", bufs=3,
                                name="wt")
                for j in range(cnt):
                    k = t0 * slabs_per_tile + j
                    eng = nc.gpsimd if USE_R else nc.sync
                    eng.dma_start(wt[:, j * ncols:(j + 1) * ncols],
                                  dram3[li, k * 128:(k + 1) * 128, :])
                tiles.append(wt)
            return lambda k: tiles[k // slabs_per_tile][
                :, (k % slabs_per_tile) * ncols:((k % slabs_per_tile) + 1) * ncols]

        def param_tile(dram3, li, name):
            t = sb.tile([128, H], F32, tag="param", bufs=4, name=name)
            nc.sync.dma_start(t[:], dram3[li])
            return t

        # ---------------- embeddings + LN ----------------
        x = sb.tile([128, NSC * H], F32, tag="x", bufs=2, name="x_emb")
        x0t = sb.tile([128, NSC * H], F32, tag="mmout", name="x0t")
        for sc in range(NSC):
            nc.sync.dma_start(x0t[:, sc * H:(sc + 1) * H],
                              x0_d[sc * 128:(sc + 1) * 128, :])
        embg = sb.tile([128, H], F32, tag="param", bufs=4, name="embg")
        nc.sync.dma_start(embg[:], embg_d[:])
        embb = sb.tile([128, H], F32, tag="param", bufs=4, name="embb")
        nc.sync.dma_start(embb[:], embb_d[:])
        for sc in range(NSC):
            ln_natural(x0t[:, sc * H:(sc + 1) * H], embg[:], embb[:],
                       x[:, sc * H:(sc + 1) * H])

        # ---------------- encoder layers ----------------
        for li in range(NL):
            wq = load_matrix(wqkv_d, li, H, 3 * HHALF, 2)       # 3 tiles
            qkb = sb.tile([128, 2 * NQK + NFFC], F32, tag="qkb", bufs=2,
                          name="qkb")
            nc.sync.dma_start(qkb[:], qkb_d[li])
            vb = sb.tile([128, HHALF], F32, tag="vb", bufs=2, name="vb")
            nc.sync.dma_start(vb[:], vb_d[li])

            xT = sb.tile([128, NHC * S], F32R if USE_R else F32, tag="xT",
                         name="xT")
            for hc in range(NHC):
                for sc in range(NSC):
                    transpose_128(
                        xT[:, hc * S + sc * 128: hc * S + (sc + 1) * 128],
                        x[:, sc * H + hc * 128: sc * H + (hc + 1) * 128])

            # --- QKV ---
            qT = sb.tile([128, NQK * S], F32, tag="qT", name="qT")
            kT = sb.tile([128, NQK * S], F32, tag="kT", name="kT")
            for qk in range(2):
                dst = qT if qk == 0 else kT
                for m in range(NQK):
                    acc = ps.tile([128, S], F32, tag="mm", name="acc_qk")
                    for k in range(NHC):
                        nc.tensor.matmul(
                            acc[:],
                            wq(k)[:, qk * HHALF + m * 128: qk * HHALF + (m + 1) * 128],
                            xT[:, k * S:(k + 1) * S],
                            start=(k == 0), stop=(k == NHC - 1))
                    nc.vector.tensor_scalar_add(
                        dst[:, m * S:(m + 1) * S], acc[:],
                        qkb[:, qk * NQK + m: qk * NQK + m + 1])
            vN = sb.tile([128, NSC * HHALF], F32, tag="vN", name="vN")
            for sc in range(NSC):
                acc = ps.tile([128, S], F32, tag="mm", name="acc_v")
                for k in range(NHC):
                    nc.tensor.matmul(
                        acc[:, :HHALF],
                        xT[:, k * S + sc * 128: k * S + (sc + 1) * 128],
                        wq(k)[:, 2 * HHALF:],
                        start=(k == 0), stop=(k == NHC - 1))
                nc.vector.tensor_add(vN[:, sc * HHALF:(sc + 1) * HHALF],
                                     acc[:, :HHALF], vb[:])

            # --- attention (6 heads) ---
            ctxT = sb.tile([128, NQK * S], F32R if USE_R else F32,
                           tag="ctxT", name="ctxT")
            for h in range(NHEADS):
                mt = h // 2
                ro = (h % 2) * 64
                qTh = qT[ro:ro + 64, mt * S:(mt + 1) * S]
                expt = sb.tile([128, NSC * S], F32, tag="exp", bufs=2,
                               name="expt")
                for kc in range(NSC):
                    sps = ps.tile([128, S], F32, tag="mm", name="sps")
                    nc.tensor.matmul(
                        sps[:],
                        kT[ro:ro + 64,
                           mt * S + kc * 128: mt * S + (kc + 1) * 128],
                        qTh, start=True, stop=True)
                    nc.scalar.activation(expt[:, kc * S:(kc + 1) * S], sps[:],
                                         AF.Exp, bias=amask[:, kc:kc + 1],
                                         scale=SCALE)
                d01 = sb.tile([128, S], F32, tag="sm", bufs=3, name="d01")
                d23 = sb.tile([128, S], F32, tag="sm", bufs=3, name="d23")
                dall = sb.tile([128, S], F32, tag="sm", bufs=3, name="dall")
                nc.vector.tensor_add(d01[:], expt[:, 0:S], expt[:, S:2 * S])
                nc.vector.tensor_add(d23[:], expt[:, 2 * S:3 * S],
                                     expt[:, 3 * S:4 * S])
                nc.vector.tensor_add(dall[:], d01[:], d23[:])
                dps = ps.tile([128, S], F32, tag="mm", name="dps")
                nc.tensor.matmul(dps[0:1, :], ones[:, 0:1], dall[:],
                                 start=True, stop=True)
                inv = stat.tile([1, S], F32, tag="inv", bufs=2, name="inv")
                nc.vector.reciprocal(inv[:], dps[0:1, :])
                bps = ps.tile([128, S], F32, tag="mm", name="bps")
                nc.tensor.matmul(bps[0:64, :], ones[0:1, 0:64], inv[:],
                                 start=True, stop=True)
                bcs = sb.tile([64, S], F32, tag="bcs", bufs=2, name="bcs")
                nc.vector.tensor_copy(bcs[:], bps[0:64, :])
                cps = ps2.tile([64, S], F32, tag="ctx", name="cps")
                for kc in range(NSC):
                    nc.tensor.matmul(
                        cps[:],
                        vN[:, kc * HHALF + h * 64: kc * HHALF + (h + 1) * 64],
                        expt[:, kc * S:(kc + 1) * S],
                        start=(kc == 0), stop=(kc == NSC - 1))
                nc.vector.tensor_mul(
                    ctxT[ro:ro + 64, mt * S:(mt + 1) * S], cps[:], bcs[:])

            # --- Wo (partial) ---
            wo = load_matrix(wo_d, li, HHALF, H, 2)              # 2 tiles
            attnP = sb.tile([128, NSC * H], F32, tag="mmout", name="attnP")
            for sc in range(NSC):
                for n in range(2):
                    nw = 512 if n == 0 else H - 512
                    acc = ps.tile([128, S], F32, tag="mm", name="acc_o")
                    for k in range(NQK):
                        nc.tensor.matmul(
                            acc[:, :nw],
                            ctxT[:, k * S + sc * 128: k * S + (sc + 1) * 128],
                            wo(k)[:, n * 512: n * 512 + nw],
                            start=(k == 0), stop=(k == NQK - 1))
                    nc.vector.tensor_copy(
                        attnP[:, sc * H + n * 512: sc * H + n * 512 + nw],
                        acc[:, :nw])

            # --- AllReduce #1 ---
            arin1 = dram.tile([S, H], F32, tag="arin", name="arin1")
            arout1 = dram.tile([S, H], F32, tag="arout", name="arout1")
            for sc in range(NSC):
                nc.sync.dma_start(arin1[sc * 128:(sc + 1) * 128, :],
                                  attnP[:, sc * H:(sc + 1) * H])
            if NO_AR:
                nc.sync.dma_start(arout1[:], arin1[:])
            else:
                nc.gpsimd.collective_compute(
                    "AllReduce", ALU.add, replica_groups=PAIRS,
                    ins=[arin1.opt()], outs=[arout1.opt()])
            attnF = sb.tile([128, NSC * H], F32, tag="mmout", name="attnF")
            for sc in range(NSC):
                nc.sync.dma_start(attnF[:, sc * H:(sc + 1) * H],
                                  arout1[sc * 128:(sc + 1) * 128, :])

            # --- residual + bias + LN1 ---
            obt = param_tile(ob_d, li, "obt")
            l1g = param_tile(ln1g_d, li, "l1g")
            l1b = param_tile(ln1b_d, li, "l1b")
            x1 = sb.tile([128, NSC * H], F32, tag="x", bufs=2, name="x1")
            for sc in range(NSC):
                t = sb.tile([128, H], F32, tag="lns", bufs=4, name="t_r1")
                nc.vector.tensor_add(t[:], attnF[:, sc * H:(sc + 1) * H], obt[:])
                r = sb.tile([128, H], F32, tag="lns", bufs=4, name="r_r1")
                nc.vector.tensor_add(r[:], t[:], x[:, sc * H:(sc + 1) * H])
                ln_natural(r[:], l1g[:], l1b[:], x1[:, sc * H:(sc + 1) * H])

            # --- x1^T ---
            x1T = sb.tile([128, NHC * S], F32R if USE_R else F32, tag="xT",
                          name="x1T")
            for hc in range(NHC):
                for sc in range(NSC):
                    transpose_128(
                        x1T[:, hc * S + sc * 128: hc * S + (sc + 1) * 128],
                        x1[:, sc * H + hc * 128: sc * H + (hc + 1) * 128])

            # --- W1 + gelu ---
            w1 = load_matrix(w1_d, li, H, FFHALF, 2)             # 3 tiles
            hT = sb.tile([128, NFFC * S], F32R if USE_R else F32, tag="hT",
                         name="hT")
            for m in range(NFFC):
                acc = ps.tile([128, S], F32, tag="mm", name="acc_h")
                for k in range(NHC):
                    nc.tensor.matmul(
                        acc[:], w1(k)[:, m * 128:(m + 1) * 128],
                        x1T[:, k * S:(k + 1) * S],
                        start=(k == 0), stop=(k == NHC - 1))
                nc.scalar.activation(
                    hT[:, m * S:(m + 1) * S], acc[:], AF.Gelu,
                    bias=qkb[:, 2 * NQK + m: 2 * NQK + m + 1])

            # --- W2 (partial) ---
            w2 = load_matrix(w2_d, li, FFHALF, H, 4)             # 3 tiles
            ffP = sb.tile([128, NSC * H], F32, tag="mmout", name="ffP")
            for sc in range(NSC):
                for n in range(2):
                    nw = 512 if n == 0 else H - 512
                    acc = ps.tile([128, S], F32, tag="mm", name="acc_f")
                    for k in range(NFFC):
                        nc.tensor.matmul(
                            acc[:, :nw],
                            hT[:, k * S + sc * 128: k * S + (sc + 1) * 128],
                            w2(k)[:, n * 512: n * 512 + nw],
                            start=(k == 0), stop=(k == NFFC - 1))
                    nc.vector.tensor_copy(
                        ffP[:, sc * H + n * 512: sc * H + n * 512 + nw],
                        acc[:, :nw])

            # --- AllReduce #2 ---
            arin2 = dram.tile([S, H], F32, tag="arin", name="arin2")
            arout2 = dram.tile([S, H], F32, tag="arout", name="arout2")
            for sc in range(NSC):
                nc.sync.dma_start(arin2[sc * 128:(sc + 1) * 128, :],
                                  ffP[:, sc * H:(sc + 1) * H])
            if NO_AR:
                nc.sync.dma_start(arout2[:], arin2[:])
            else:
                nc.gpsimd.collective_compute(
                    "AllReduce", ALU.add, replica_groups=PAIRS,
                    ins=[arin2.opt()], outs=[arout2.opt()])
            ffF = sb.tile([128, NSC * H], F32, tag="mmout", name="ffF")
            for sc in range(NSC):
                nc.sync.dma_start(ffF[:, sc * H:(sc + 1) * H],
                                  arout2[sc * 128:(sc + 1) * 128, :])

            # --- residual + bias + LN2 ---
            b2t = param_tile(b2_d, li, "b2t")
            l2g = param_tile(ln2g_d, li, "l2g")
            l2b = param_tile(ln2b_d, li, "l2b")
            x_next = sb.tile([128, NSC * H], F32, tag="x", bufs=2, name="x_n")
            for sc in range(NSC):
                t = sb.tile([128, H], F32, tag="lns", bufs=4, name="t_r2")
                nc.vector.tensor_add(t[:], ffF[:, sc * H:(sc + 1) * H], b2t[:])
                r = sb.tile([128, H], F32, tag="lns", bufs=4, name="r_r2")
                nc.vector.tensor_add(r[:], t[:], x1[:, sc * H:(sc + 1) * H])
                ln_natural(r[:], l2g[:], l2b[:], x_next[:, sc * H:(sc + 1) * H])
            x = x_next

        # ---------------- final pairwise-distance block ----------------
        xTf = sb.tile([128, NHC * S], F32, tag="xT", name="xTf")
        for hc in range(NHC):
            for sc in range(NSC):
                transpose_128(
                    xTf[:, hc * S + sc * 128: hc * S + (sc + 1) * 128],
                    x[:, sc * H + hc * 128: sc * H + (hc + 1) * 128])

        # our query columns of x^T (half g, blended via hsel one-hot)
        xqT = sb.tile([128, NHC * SQ], F32, tag="qT", name="xqT")
        for hc in range(NHC):
            t0 = sb.tile([128, SQ], F32, tag="sm", bufs=3, name="t0sel")
            nc.vector.tensor_scalar_mul(t0[:], xTf[:, hc * S: hc * S + SQ],
                                        hsel[:, 0:1])
            nc.vector.scalar_tensor_tensor(
                xqT[:, hc * SQ:(hc + 1) * SQ],
                xTf[:, hc * S + SQ: hc * S + S], hsel[:, 1:2], t0[:],
                op0=ALU.mult, op1=ALU.add)

        # sq_k [1, S] then broadcast [128, S]
        sqk_ps = ps.tile([128, S], F32, tag="mm", name="sqk_ps")
        for hc in range(NHC):
            sqt = sb.tile([128, S], F32, tag="sm", bufs=3, name="sqt")
            nc.scalar.activation(sqt[:], xTf[:, hc * S:(hc + 1) * S], AF.Square)
            nc.tensor.matmul(sqk_ps[0:1, :], ones[:, 0:1], sqt[:],
                             start=(hc == 0), stop=(hc == NHC - 1))
        sqk = stat.tile([1, S], F32, tag="inv", bufs=2, name="sqk")
        nc.vector.tensor_copy(sqk[:], sqk_ps[0:1, :])
        bps_f = ps.tile([128, S], F32, tag="mm", name="bps_f")
        nc.tensor.matmul(bps_f[:], ones[0:1, :], sqk[:], start=True, stop=True)
        finb = sb.tile([128, NQK * S], F32, tag="kT", name="finb")
        sqk_bc = finb[:, 0:S]
        nc.vector.tensor_copy(sqk_bc, bps_f[:])

        # sq_q per q-chunk [128, 1]: square+accum on x chunks, blend by hsel
        sqx = []
        for sc in range(NSC):
            scr = sb.tile([128, H], F32, tag="lns", bufs=4, name="scr_sq")
            acc = stat.tile([128, 1], F32, tag="stat", name="sqx_acc")
            nc.scalar.activation(scr[:], x[:, sc * H:(sc + 1) * H], AF.Square,
                                 accum_out=acc[:])
            sqx.append(acc)
        sqq = []
        for qc in range(2):
            s0 = stat.tile([128, 1], F32, tag="stat", name="s0_sel")
            nc.vector.tensor_scalar_mul(s0[:], sqx[qc][:], hsel[:, 0:1])
            sq_ = stat.tile([128, 1], F32, tag="stat", name="sq_sel")
            nc.vector.scalar_tensor_tensor(sq_[:], sqx[2 + qc][:], hsel[:, 1:2],
                                           s0[:], op0=ALU.mult, op1=ALU.add)
            sqq.append(sq_)

        # op embeddings
        opt_t = sb.tile([128, NHC * NOPS], F32, tag="opT", name="opt_t")
        for hc in range(NHC):
            nc.sync.dma_start(opt_t[:, hc * NOPS:(hc + 1) * NOPS],
                              opT_d[hc * 128:(hc + 1) * 128, :])
        sqo_ps = ps.tile([128, S], F32, tag="mm", name="sqo_ps")
        for hc in range(NHC):
            sqt2 = sb.tile([128, NOPS], F32, tag="sqot", bufs=2, name="sqt2")
            nc.scalar.activation(sqt2[:], opt_t[:, hc * NOPS:(hc + 1) * NOPS],
                                 AF.Square)
            nc.tensor.matmul(sqo_ps[0:1, :NOPS], ones[:, 0:1], sqt2[:],
                             start=(hc == 0), stop=(hc == NHC - 1))
        sqo = stat.tile([1, NOPS], F32, tag="sqo", bufs=2, name="sqo")
        nc.vector.tensor_copy(sqo[:], sqo_ps[0:1, :NOPS])
        bps_o = ps.tile([128, S], F32, tag="mm", name="bps_o")
        nc.tensor.matmul(bps_o[:, :NOPS], ones[0:1, :], sqo[:],
                         start=True, stop=True)
        sqo_bc = sb.tile([128, NOPS], F32, tag="sqobc", name="sqo_bc")
        nc.vector.tensor_copy(sqo_bc[:], bps_o[:, :NOPS])

        dist_t, disto_t = [], []
        for qc in range(2):
            gps = ps.tile([128, S], F32, tag="mm", name="gps")
            for hc in range(NHC):
                nc.tensor.matmul(
                    gps[:], xqT[:, hc * SQ + qc * 128: hc * SQ + (qc + 1) * 128],
                    xTf[:, hc * S:(hc + 1) * S],
                    start=(hc == 0), stop=(hc == NHC - 1))
            d2 = sb.tile([128, S], F32, tag="sm", bufs=3, name="d2")
            nc.vector.scalar_tensor_tensor(d2[:], gps[:], -2.0, sqk_bc[:],
                                           op0=ALU.mult, op1=ALU.add)
            d2b = sb.tile([128, S], F32, tag="sm", bufs=3, name="d2b")
            nc.vector.tensor_scalar(d2b[:], d2[:], sqq[qc][:], 0.0,
                                    op0=ALU.add, op1=ALU.max)
            dt = finb[:, (1 + qc) * S:(2 + qc) * S]
            nc.scalar.activation(dt, d2b[:], AF.Sqrt)
            dist_t.append(dt)

            gops = ps.tile([128, S], F32, tag="mm", name="gops")
            for hc in range(NHC):
                nc.tensor.matmul(
                    gops[:, :NOPS],
                    xqT[:, hc * SQ + qc * 128: hc * SQ + (qc + 1) * 128],
                    opt_t[:, hc * NOPS:(hc + 1) * NOPS],
                    start=(hc == 0), stop=(hc == NHC - 1))
            d2o = sb.tile([128, NOPS], F32, tag="disto", bufs=6, name="d2o")
            nc.vector.scalar_tensor_tensor(d2o[:], gops[:, :NOPS], -2.0,
                                           sqo_bc[:], op0=ALU.mult, op1=ALU.add)
            d2ob = sb.tile([128, NOPS], F32, tag="disto", bufs=6, name="d2ob")
            nc.vector.tensor_scalar(d2ob[:], d2o[:], sqq[qc][:], 0.0,
                                    op0=ALU.add, op1=ALU.max)
            dto = sb.tile([128, NOPS], F32, tag="disto", bufs=6, name="dto")
            nc.scalar.activation(dto[:], d2ob[:], AF.Sqrt)
            disto_t.append(dto)

        # partial sums -> [1, 2] -> global AllReduce -> means
        r0 = stat.tile([128, 1], F32, tag="stat", name="r0")
        r1 = stat.tile([128, 1], F32, tag="stat", name="r1")
        rsum = stat.tile([128, 1], F32, tag="stat", name="rsum")
        nc.vector.tensor_reduce(r0[:], dist_t[0], axis=mybir.AxisListType.X,
                                op=ALU.add)
        nc.vector.tensor_reduce(r1[:], dist_t[1], axis=mybir.AxisListType.X,
                                op=ALU.add)
        nc.vector.tensor_add(rsum[:], r0[:], r1[:])
        ro0 = stat.tile([128, 1], F32, tag="stat", name="ro0")
        ro1 = stat.tile([128, 1], F32, tag="stat", name="ro1")
        rosum = stat.tile([128, 1], F32, tag="stat", name="rosum")
        nc.vector.tensor_reduce(ro0[:], disto_t[0][:],
                                axis=mybir.AxisListType.X, op=ALU.add)
        nc.vector.tensor_reduce(ro1[:], disto_t[1][:],
                                axis=mybir.AxisListType.X, op=ALU.add)
        nc.vector.tensor_add(rosum[:], ro0[:], ro1[:])
        tots = sb.tile([1, 2], F32, tag="tots", bufs=2, name="tots")
        tps_a = ps.tile([128, S], F32, tag="mm", name="tps_a")
        nc.tensor.matmul(tps_a[0:1, 0:1], ones[:, 0:1], rsum[:],
                         start=True, stop=True)
        nc.vector.tensor_copy(tots[:, 0:1], tps_a[0:1, 0:1])
        tps_b = ps.tile([128, S], F32, tag="mm", name="tps_b")
        nc.tensor.matmul(tps_b[0:1, 0:1], ones[:, 0:1], rosum[:],
                         start=True, stop=True)
        nc.vector.tensor_copy(tots[:, 1:2], tps_b[0:1, 0:1])

        arin3 = dram.tile([1, 2], F32, tag="arin3", name="arin3")
        arout3 = dram.tile([1, 2], F32, tag="arout3", name="arout3")
        nc.sync.dma_start(arin3[:], tots[:])
        nc.gpsimd.collective_compute(
            "AllReduce", ALU.add, replica_groups=[list(range(N_CORES))],
            ins=[arin3.opt()], outs=[arout3.opt()])
        ar3 = sb.tile([1, 2], F32, tag="tots", bufs=2, name="ar3")
        nc.sync.dma_start(ar3[:], arout3[:])
        means = stat.tile([1, 2], F32, tag="sqo", bufs=2, name="means")
        nc.vector.tensor_mul(means[:], ar3[:], scal[:])
        mb_ps = ps.tile([128, S], F32, tag="mm", name="mb_ps")
        nc.tensor.matmul(mb_ps[:, 0:2], ones[0:1, :], means[:],
                         start=True, stop=True)
        mean_bc = sb.tile([128, 2], F32, tag="meanbc", name="mean_bc")
        nc.vector.tensor_copy(mean_bc[:], mb_ps[:, 0:2])

        sigblk = sb.tile([128, NSC * H], F32, tag="mmout", name="sigblk")
        for qc in range(2):
            sig = sigblk[:, 0:S] if qc == 0 else sigblk[:, 3 * S:4 * S]
            nc.scalar.activation(sig, dist_t[qc], AF.Sigmoid,
                                 bias=mean_bc[:, 0:1], scale=-1.0)
            msk = sigblk[:, S:2 * S] if qc == 0 else sigblk[:, 4 * S:5 * S]
            nc.vector.tensor_scalar(msk, sig, THRESH, None, op0=ALU.is_ge)
            ww = sigblk[:, 2 * S:3 * S] if qc == 0 else sigblk[:, 5 * S:6 * S]
            nc.vector.tensor_mul(ww, sig, msk)
            nc.sync.dma_start(ww_out[qc * 128:(qc + 1) * 128, :], ww)

            sigo = sb.tile([128, NOPS], F32, tag="disto", bufs=6, name="sigo")
            nc.scalar.activation(sigo[:], disto_t[qc][:], AF.Sigmoid,
                                 bias=mean_bc[:, 1:2], scale=-1.0)
            msko = sb.tile([128, NOPS], F32, tag="disto", bufs=6, name="msko")
            nc.vector.tensor_scalar(msko[:], sigo[:], THRESH, None,
                                    op0=ALU.is_ge)
            wwo = sb.tile([128, NOPS], F32, tag="disto", bufs=6, name="wwo")
            nc.vector.tensor_mul(wwo[:], sigo[:], msko[:])
            nc.sync.dma_start(wop_out[qc * 128:(qc + 1) * 128, :], wwo[:])

        # embedded output (our half rows, chunk = 2*g + qc, blended by hsel)
        for qc in range(2):
            t0e = sb.tile([128, H], F32, tag="lns", bufs=4, name="t0e")
            nc.vector.tensor_scalar_mul(t0e[:], x[:, qc * H:(qc + 1) * H],
                                        hsel[:, 0:1])
            emb_sel = sb.tile([128, H], F32, tag="lns", bufs=4, name="emb_sel")
            nc.vector.scalar_tensor_tensor(
                emb_sel[:], x[:, (2 + qc) * H:(3 + qc) * H], hsel[:, 1:2],
                t0e[:], op0=ALU.mult, op1=ALU.add)
            nc.sync.dma_start(emb_out[qc * 128:(qc + 1) * 128, :], emb_sel[:])

    nc.finalize()
    return nc


# ---------------------------------------------------------------------------
# Host side: input prep, runner, output assembly
# ---------------------------------------------------------------------------

_RUNNER = None


class _Runner:
    def __init__(self):
        self.nc = build_program()
        nc = self.nc
        bass2jax.install_neuronx_cc_hook()
        partition_name = (nc.partition_id_tensor.name
                          if nc.partition_id_tensor else None)
        in_names, out_names, out_avals, zero_outs = [], [], [], []
        for alloc in nc.m.functions[0].allocations:
            if not isinstance(alloc, mybir.MemoryLocationSet):
                continue
            name = alloc.memorylocations[0].name
            if alloc.kind == "ExternalInput":
                if name != partition_name:
                    in_names.append(name)
            elif alloc.kind == "ExternalOutput":
                out_names.append(name)
                shape = tuple(alloc.tensor_shape)
                dtype = mybir.dt.np(alloc.dtype)
                out_avals.append(jax.core.ShapedArray(shape, dtype))
                zero_outs.append(np.zeros(shape, dtype))
        self.n_params = len(in_names)
        self.param_names = list(in_names)
        self.out_names = out_names
        self.out_avals = out_avals
        self.zero_outs = zero_outs
        all_in = in_names + out_names
        if partition_name:
            all_in.append(partition_name)

        def _body(*args):
            operands = list(args)
            if partition_name:
                operands.append(bass2jax.partition_id_tensor())
            outs = bass2jax._bass_exec_p.bind(
                *operands, out_avals=tuple(out_avals),
                in_names=tuple(all_in), out_names=tuple(out_names),
                lowering_input_output_aliases=(), sim_require_finite=True,
                sim_require_nnan=True, nc=nc)
            return tuple(outs)

        devices = jax.devices()[:N_CORES]
        self.mesh = Mesh(np.asarray(devices), ("core",))
        n_in = self.n_params + len(out_names)
        self.sharded = jax.jit(
            shard_map(_body, mesh=self.mesh,
                      in_specs=(PartitionSpec("core"),) * n_in,
                      out_specs=(PartitionSpec("core"),) * len(out_names),
                      check_rep=False),
            keep_unused=True)
        self.sharding = NamedSharding(self.mesh, PartitionSpec("core"))

    def place(self, per_core_maps):
        args = []
        for name in self.param_names:
            cat = np.concatenate([np.asarray(m[name]) for m in per_core_maps],
                                 axis=0)
            args.append(jax.device_put(cat, self.sharding))
        for z in self.zero_outs:
            cat = np.zeros((N_CORES * z.shape[0], *z.shape[1:]), z.dtype)
            args.append(jax.device_put(cat, self.sharding))
        return args

    def run_raw(self, args):
        return self.sharded(*args)

    def run(self, args):
        outs = self.sharded(*args)
        jax.block_until_ready(outs)
        return {name: np.asarray(o).reshape(N_CORES, *self.out_avals[i].shape)
                for i, (name, o) in enumerate(
                    zip(self.out_names, outs))}


def get_runner():
    global _RUNNER
    if _RUNNER is None:
        _RUNNER = _Runner()
    return _RUNNER


def prepare_core_inputs(inputs, core):
    p, g = core // 2, core % 2
    f32 = np.float32
    ids = np.asarray(inputs["input_ids"])[p]
    x0 = (np.asarray(inputs["word_emb"], f32)[ids]
          + np.asarray(inputs["pos_emb"], f32)[:S]
          + np.asarray(inputs["type_emb"], f32)[0][None, :]).astype(f32)
    am = np.asarray(inputs["attention_mask"])[p].astype(f32)
    amask = np.ascontiguousarray(
        ((1.0 - am) * -1e9).astype(f32).reshape(NSC, 128).T)

    rep = lambda v: np.repeat(np.asarray(v, f32).reshape(1, -1), 128, axis=0)
    embg = rep(inputs["emb_ln_g"])
    embb = rep(inputs["emb_ln_b"])

    Wqkv = np.asarray(inputs["Wqkv"], f32)
    bqkv = np.asarray(inputs["bqkv"], f32)
    Wo = np.asarray(inputs["Wo"], f32)
    bo = np.asarray(inputs["bo"], f32)
    W1 = np.asarray(inputs["W1"], f32)
    b1 = np.asarray(inputs["b1"], f32)
    W2 = np.asarray(inputs["W2"], f32)
    b2 = np.asarray(inputs["b2"], f32)

    qs = slice(g * HHALF, (g + 1) * HHALF)
    ks = slice(H + g * HHALF, H + (g + 1) * HHALF)
    vs = slice(2 * H + g * HHALF, 2 * H + (g + 1) * HHALF)
    wqkv = np.ascontiguousarray(
        np.concatenate([Wqkv[:, :, qs], Wqkv[:, :, ks], Wqkv[:, :, vs]],
                       axis=2))[:NL]
    wo = np.ascontiguousarray(Wo[:, g * HHALF:(g + 1) * HHALF, :])[:NL]
    w1 = np.ascontiguousarray(W1[:, :, g * FFHALF:(g + 1) * FFHALF])[:NL]
    w2 = np.ascontiguousarray(W2[:, g * FFHALF:(g + 1) * FFHALF, :])[:NL]

    nl = NL
    qkb = np.zeros((nl, 128, 2 * NQK + NFFC), f32)
    for m in range(NQK):
        qkb[:, :, m] = bqkv[:nl, g * HHALF + m * 128: g * HHALF + (m + 1) * 128]
        qkb[:, :, NQK + m] = bqkv[:nl, H + g * HHALF + m * 128:
                                  H + g * HHALF + (m + 1) * 128]
    for m in range(NFFC):
        qkb[:, :, 2 * NQK + m] = b1[:nl, g * FFHALF + m * 128:
                                    g * FFHALF + (m + 1) * 128]
    vb = np.repeat(bqkv[:nl, None, 2 * H + g * HHALF: 2 * H + (g + 1) * HHALF],
                   128, axis=1).astype(f32)
    repl = lambda a: np.repeat(np.asarray(a, f32)[:nl, None, :], 128, axis=1)
    ob = repl(bo)
    b2r = repl(b2)
    ln1g = repl(inputs["ln1_g"])
    ln1b = repl(inputs["ln1_b"])
    ln2g = repl(inputs["ln2_g"])
    ln2b = repl(inputs["ln2_b"])

    opT = np.ascontiguousarray(np.asarray(inputs["op_emb"], f32).T)
    scal = np.array([[1.0 / (B * S * S), 1.0 / (B * S * NOPS)]], f32)
    hsel = np.zeros((128, 2), f32)
    hsel[:, g] = 1.0

    return {
        "x0": x0, "amask": amask, "embg": embg, "embb": embb,
        "wqkv": wqkv, "wo": wo, "w1": w1, "w2": w2,
        "qkb": qkb, "vb": vb, "ob": ob, "b2": b2r,
        "ln1g": ln1g, "ln1b": ln1b, "ln2g": ln2g, "ln2b": ln2b,
        "opT": opT, "scal": scal, "hsel": hsel,
    }


def assemble_outputs(res):
    f32 = np.float32
    embedded = np.zeros((B, S, H), f32)
    word_word = np.zeros((B, S, S), f32)
    word_operator = np.zeros((B, S, NOPS), f32)
    for c in range(N_CORES):
        p, g = c // 2, c % 2
        rows = slice(g * SQ, (g + 1) * SQ)
        embedded[p, rows] = res["emb_out"][c]
        word_word[p, rows] = res["ww_out"][c]
        word_operator[p, rows] = res["wop_out"][c]
    return embedded, word_word, word_operator


def kernel(**inputs):
    r = get_runner()
    maps = [prepare_core_inputs(inputs, c) for c in range(N_CORES)]
    args = r.place(maps)
    res = r.run(args)
    return assemble_outputs(res)


# revision 10
# speedup vs baseline: 1.0320x; 1.0320x over previous
"""Trainium2 Bass kernel for nn_BertEncoder_45260365365543.

BERT-base encoder (12 layers, B=4, S=512, H=768) + pairwise L2-distance
outputs, on 8 NeuronCores:
  - 4 pairs of cores; pair p handles batch element p.
  - Within a pair: tensor-parallel split (6 of 12 heads, half of the FFN
    per core) with two pairwise AllReduces per layer (after the Wo partial
    and after the W2 partial).
  - Final pairwise-distance block: both cores of a pair hold the full
    embedded sequence after the last AllReduce; each computes its half of
    the query rows of word_word / word_operator locally. One global
    AllReduce produces the distance means.

All compute is fp32 (PE fp32 matmuls, fp32 PSUM accumulation) to keep the
sigmoid-threshold outputs numerically faithful to the fp32 reference.
"""

import os
import sys

sys.path.insert(0, "/opt/trn_rl_repo")

import numpy as np

import concourse.bass as bass
import concourse.mybir as mybir
import concourse.tile as tile
from concourse import bacc, bass2jax
from concourse.masks import make_identity

import jax
from jax.sharding import Mesh, PartitionSpec, NamedSharding
from jax.experimental.shard_map import shard_map

F32 = mybir.dt.float32
AF = mybir.ActivationFunctionType
ALU = mybir.AluOpType

NL = int(os.environ.get("BK_NL", "12"))
USE_R = os.environ.get("BK_R", "0") == "1"
NO_AR = os.environ.get("BK_NOAR", "0") == "1"   # timing ablation only
F32R = mybir.dt.float32r
H, NH, DH, FF, S, B = 768, 12, 64, 3072, 512, 4
NOPS = 16
THRESH = 0.4
EPS = 1e-12
SCALE = 1.0 / float(np.sqrt(DH))

HHALF = H // 2            # 384 features (6 heads) per core
FFHALF = FF // 2          # 1536
NSC = S // 128            # 4 sequence chunks
NHC = H // 128            # 6 feature chunks
NFFC = FFHALF // 128      # 12 ff chunks per core
NQK = HHALF // 128        # 3 chunks for Q^T / K^T halves
NHEADS = NH // 2          # 6 heads per core
SQ = S // 2               # query rows handled per core

N_CORES = 8
PAIRS = [[2 * p, 2 * p + 1] for p in range(4)]


def build_program():
    nc = bacc.Bacc()

    # ---------------- DRAM I/O ----------------
    x0_d = nc.dram_tensor("x0", [S, H], F32, kind="ExternalInput")
    amask_d = nc.dram_tensor("amask", [128, NSC], F32, kind="ExternalInput")
    embg_d = nc.dram_tensor("embg", [128, H], F32, kind="ExternalInput")
    embb_d = nc.dram_tensor("embb", [128, H], F32, kind="ExternalInput")
    wqkv_d = nc.dram_tensor("wqkv", [NL, H, 3 * HHALF], F32, kind="ExternalInput")
    wo_d = nc.dram_tensor("wo", [NL, HHALF, H], F32, kind="ExternalInput")
    w1_d = nc.dram_tensor("w1", [NL, H, FFHALF], F32, kind="ExternalInput")
    w2_d = nc.dram_tensor("w2", [NL, FFHALF, H], F32, kind="ExternalInput")
    qkb_d = nc.dram_tensor("qkb", [NL, 128, 2 * NQK + NFFC], F32,
                           kind="ExternalInput")
    vb_d = nc.dram_tensor("vb", [NL, 128, HHALF], F32, kind="ExternalInput")
    ob_d = nc.dram_tensor("ob", [NL, 128, H], F32, kind="ExternalInput")
    b2_d = nc.dram_tensor("b2", [NL, 128, H], F32, kind="ExternalInput")
    ln1g_d = nc.dram_tensor("ln1g", [NL, 128, H], F32, kind="ExternalInput")
    ln1b_d = nc.dram_tensor("ln1b", [NL, 128, H], F32, kind="ExternalInput")
    ln2g_d = nc.dram_tensor("ln2g", [NL, 128, H], F32, kind="ExternalInput")
    ln2b_d = nc.dram_tensor("ln2b", [NL, 128, H], F32, kind="ExternalInput")
    opT_d = nc.dram_tensor("opT", [H, NOPS], F32, kind="ExternalInput")
    scal_d = nc.dram_tensor("scal", [1, 2], F32, kind="ExternalInput")
    hsel_d = nc.dram_tensor("hsel", [128, 2], F32, kind="ExternalInput")

    emb_out = nc.dram_tensor("emb_out", [SQ, H], F32, kind="ExternalOutput")
    ww_out = nc.dram_tensor("ww_out", [SQ, S], F32, kind="ExternalOutput")
    wop_out = nc.dram_tensor("wop_out", [SQ, NOPS], F32, kind="ExternalOutput")

    from contextlib import ExitStack
    with tile.TileContext(nc) as tc, ExitStack() as ctx:
        const = ctx.enter_context(tc.tile_pool(name="const", bufs=1))
        sb = ctx.enter_context(tc.tile_pool(name="sb", bufs=1))
        stat = ctx.enter_context(tc.tile_pool(name="stat", bufs=12))
        wpool = ctx.enter_context(tc.tile_pool(name="wpool", bufs=4))
        ps = ctx.enter_context(tc.tile_pool(name="ps", bufs=4, space="PSUM"))
        ps2 = ctx.enter_context(tc.tile_pool(name="ps2", bufs=2, space="PSUM"))
        dram = ctx.enter_context(tc.tile_pool(name="dram", bufs=1, space="DRAM"))

        ones = const.tile([128, 128], F32)
        nc.gpsimd.memset(ones[:], 1.0)
        ident = const.tile([128, 128], F32)
        make_identity(nc, ident[:])
        amask = const.tile([128, NSC], F32)
        nc.sync.dma_start(amask[:], amask_d[:])
        scal = const.tile([1, 2], F32)
        nc.sync.dma_start(scal[:], scal_d[:])
        hsel = const.tile([128, 2], F32)
        nc.sync.dma_start(hsel[:], hsel_d[:])
        epst = const.tile([128, 1], F32)
        nc.gpsimd.memset(epst[:], EPS)

        def transpose_128(out_slice, in_slice):
            tp = ps2.tile([128, 128], F32, tag="tp", name="tp")
            nc.tensor.transpose(tp[:], in_slice, ident[:])
            nc.vector.tensor_copy(out_slice, tp[:])

        def ln_natural(x_slice, g_tile, b_tile, out_slice):
            """LayerNorm along the free dim of a [128, H] slice."""
            mus = stat.tile([128, 1], F32, tag="stat", name="mus")
            nc.vector.tensor_reduce(mus[:], x_slice, axis=mybir.AxisListType.X,
                                    op=ALU.add)
            mu = stat.tile([128, 1], F32, tag="stat", name="mu")
            nc.scalar.mul(mu[:], mus[:], 1.0 / H)
            xc = sb.tile([128, H], F32, tag="lns", bufs=4, name="xc")
            nc.vector.tensor_scalar_sub(xc[:], x_slice, mu[:])
            sq = sb.tile([128, H], F32, tag="lns", bufs=4, name="sq")
            ss = stat.tile([128, 1], F32, tag="stat", name="ss")
            nc.scalar.activation(sq[:], xc[:], AF.Square, accum_out=ss[:])
            std = stat.tile([128, 1], F32, tag="stat", name="std")
            nc.scalar.activation(std[:], ss[:], AF.Sqrt, bias=epst[:], scale=1.0 / H)
            istd = stat.tile([128, 1], F32, tag="stat", name="istd")
            nc.vector.reciprocal(istd[:], std[:])
            t2 = sb.tile([128, H], F32, tag="lns", bufs=4, name="t2")
            nc.vector.scalar_tensor_tensor(t2[:], xc[:], istd[:], g_tile,
                                           op0=ALU.mult, op1=ALU.mult)
            nc.vector.tensor_add(out_slice, t2[:], b_tile)

        def load_matrix(dram3, li, nrows, ncols, slabs_per_tile):
            """Load [nrows, ncols] matrix (layer li) into tiles of
            slabs_per_tile row-slabs each; returns slab accessor."""
            nslab = nrows // 128
            tiles = []
            for t0 in range((nslab + slabs_per_tile - 1) // slabs_per_tile):
                cnt = min(slabs_per_tile, nslab - t0 * slabs_per_tile)
                wdt = F32R if USE_R else F32
                wt = wpool.tile([128, cnt * ncols], wdt, tag="wb", bufs=3,
                                name="wt")
                for j in range(cnt):
                    k = t0 * slabs_per_tile + j
                    eng = nc.gpsimd if USE_R else nc.sync
                    eng.dma_start(wt[:, j * ncols:(j + 1) * ncols],
                                  dram3[li, k * 128:(k + 1) * 128, :])
                tiles.append(wt)
            return lambda k: tiles[k // slabs_per_tile][
                :, (k % slabs_per_tile) * ncols:((k % slabs_per_tile) + 1) * ncols]

        def param_tile(dram3, li, name):
            t = sb.tile([128, H], F32, tag="param", bufs=4, name=name)
            nc.sync.dma_start(t[:], dram3[li])
            return t

        # ---------------- embeddings + LN ----------------
        x = sb.tile([128, NSC * H], F32, tag="x", bufs=2, name="x_emb")
        x0t = sb.tile([128, NSC * H], F32, tag="mmout", name="x0t")
        for sc in range(NSC):
            nc.sync.dma_start(x0t[:, sc * H:(sc + 1) * H],
                              x0_d[sc * 128:(sc + 1) * 128, :])
        embg = sb.tile([128, H], F32, tag="param", bufs=4, name="embg")
        nc.sync.dma_start(embg[:], embg_d[:])
        embb = sb.tile([128, H], F32, tag="param", bufs=4, name="embb")
        nc.sync.dma_start(embb[:], embb_d[:])
        for sc in range(NSC):
            ln_natural(x0t[:, sc * H:(sc + 1) * H], embg[:], embb[:],
                       x[:, sc * H:(sc + 1) * H])

        # ---------------- encoder layers ----------------
        for li in range(NL):
            wq = load_matrix(wqkv_d, li, H, 3 * HHALF, 2)       # 3 tiles
            qkb = sb.tile([128, 2 * NQK + NFFC], F32, tag="qkb", bufs=2,
                          name="qkb")
            nc.sync.dma_start(qkb[:], qkb_d[li])
            vb = sb.tile([128, HHALF], F32, tag="vb", bufs=2, name="vb")
            nc.sync.dma_start(vb[:], vb_d[li])

            xT = sb.tile([128, NHC * S], F32R if USE_R else F32, tag="xT",
                         name="xT")
            for hc in range(NHC):
                for sc in range(NSC):
                    transpose_128(
                        xT[:, hc * S + sc * 128: hc * S + (sc + 1) * 128],
                        x[:, sc * H + hc * 128: sc * H + (hc + 1) * 128])

            # --- QKV ---
            qT = sb.tile([128, NQK * S], F32, tag="qT", name="qT")
            kT = sb.tile([128, NQK * S], F32, tag="kT", name="kT")
            for qk in range(2):
                dst = qT if qk == 0 else kT
                for m in range(NQK):
                    acc = ps.tile([128, S], F32, tag="mm", name="acc_qk")
                    for k in range(NHC):
                        nc.tensor.matmul(
                            acc[:],
                            wq(k)[:, qk * HHALF + m * 128: qk * HHALF + (m + 1) * 128],
                            xT[:, k * S:(k + 1) * S],
                            start=(k == 0), stop=(k == NHC - 1))
                    nc.scalar.activation(dst[:, m * S:(m + 1) * S], acc[:],
                                         AF.Identity,
                                         bias=qkb[:, qk * NQK + m: qk * NQK + m + 1])
            vN = sb.tile([128, NSC * HHALF], F32, tag="vN", name="vN")
            for sc in range(NSC):
                acc = ps.tile([128, S], F32, tag="mm", name="acc_v")
                for k in range(NHC):
                    nc.tensor.matmul(
                        acc[:, :HHALF],
                        xT[:, k * S + sc * 128: k * S + (sc + 1) * 128],
                        wq(k)[:, 2 * HHALF:],
                        start=(k == 0), stop=(k == NHC - 1))
                nc.vector.tensor_add(vN[:, sc * HHALF:(sc + 1) * HHALF],
                                     acc[:, :HHALF], vb[:])

            # --- attention (6 heads) ---
            ctxT = sb.tile([128, NQK * S], F32R if USE_R else F32,
                           tag="ctxT", name="ctxT")
            for h in range(NHEADS):
                mt = h // 2
                ro = (h % 2) * 64
                qTh = qT[ro:ro + 64, mt * S:(mt + 1) * S]
                expt = sb.tile([128, NSC * S], F32, tag="exp", bufs=2,
                               name="expt")
                for kc in range(NSC):
                    sps = ps.tile([128, S], F32, tag="mm", name="sps")
                    nc.tensor.matmul(
                        sps[:],
                        kT[ro:ro + 64,
                           mt * S + kc * 128: mt * S + (kc + 1) * 128],
                        qTh, start=True, stop=True)
                    nc.scalar.activation(expt[:, kc * S:(kc + 1) * S], sps[:],
                                         AF.Exp, bias=amask[:, kc:kc + 1],
                                         scale=SCALE)
                d01 = sb.tile([128, S], F32, tag="sm", bufs=3, name="d01")
                d23 = sb.tile([128, S], F32, tag="sm", bufs=3, name="d23")
                dall = sb.tile([128, S], F32, tag="sm", bufs=3, name="dall")
                nc.vector.tensor_add(d01[:], expt[:, 0:S], expt[:, S:2 * S])
                nc.vector.tensor_add(d23[:], expt[:, 2 * S:3 * S],
                                     expt[:, 3 * S:4 * S])
                nc.vector.tensor_add(dall[:], d01[:], d23[:])
                dps = ps.tile([128, S], F32, tag="mm", name="dps")
                nc.tensor.matmul(dps[0:1, :], ones[:, 0:1], dall[:],
                                 start=True, stop=True)
                inv = stat.tile([1, S], F32, tag="inv", bufs=2, name="inv")
                nc.vector.reciprocal(inv[:], dps[0:1, :])
                bps = ps.tile([128, S], F32, tag="mm", name="bps")
                nc.tensor.matmul(bps[0:64, :], ones[0:1, 0:64], inv[:],
                                 start=True, stop=True)
                bcs = sb.tile([64, S], F32, tag="bcs", bufs=2, name="bcs")
                nc.scalar.copy(bcs[:], bps[0:64, :])
                cps = ps2.tile([64, S], F32, tag="ctx", name="cps")
                for kc in range(NSC):
                    nc.tensor.matmul(
                        cps[:],
                        vN[:, kc * HHALF + h * 64: kc * HHALF + (h + 1) * 64],
                        expt[:, kc * S:(kc + 1) * S],
                        start=(kc == 0), stop=(kc == NSC - 1))
                nc.vector.tensor_mul(
                    ctxT[ro:ro + 64, mt * S:(mt + 1) * S], cps[:], bcs[:])

            # --- Wo (partial) ---
            wo = load_matrix(wo_d, li, HHALF, H, 2)              # 2 tiles
            attnP = sb.tile([128, NSC * H], F32, tag="mmout", name="attnP")
            for sc in range(NSC):
                for n in range(2):
                    nw = 512 if n == 0 else H - 512
                    acc = ps.tile([128, S], F32, tag="mm", name="acc_o")
                    for k in range(NQK):
                        nc.tensor.matmul(
                            acc[:, :nw],
                            ctxT[:, k * S + sc * 128: k * S + (sc + 1) * 128],
                            wo(k)[:, n * 512: n * 512 + nw],
                            start=(k == 0), stop=(k == NQK - 1))
                    nc.scalar.copy(
                        attnP[:, sc * H + n * 512: sc * H + n * 512 + nw],
                        acc[:, :nw])

            # --- AllReduce #1 ---
            arin1 = dram.tile([S, H], F32, tag="arin", name="arin1")
            arout1 = dram.tile([S, H], F32, tag="arout", name="arout1")
            for sc in range(NSC):
                nc.sync.dma_start(arin1[sc * 128:(sc + 1) * 128, :],
                                  attnP[:, sc * H:(sc + 1) * H])
            if NO_AR:
                nc.sync.dma_start(arout1[:], arin1[:])
            else:
                nc.gpsimd.collective_compute(
                    "AllReduce", ALU.add, replica_groups=PAIRS,
                    ins=[arin1.opt()], outs=[arout1.opt()])
            attnF = sb.tile([128, NSC * H], F32, tag="mmout", name="attnF")
            for sc in range(NSC):
                nc.sync.dma_start(attnF[:, sc * H:(sc + 1) * H],
                                  arout1[sc * 128:(sc + 1) * 128, :])

            # --- residual + bias + LN1 ---
            obt = param_tile(ob_d, li, "obt")
            l1g = param_tile(ln1g_d, li, "l1g")
            l1b = param_tile(ln1b_d, li, "l1b")
            x1 = sb.tile([128, NSC * H], F32, tag="x", bufs=2, name="x1")
            for sc in range(NSC):
                t = sb.tile([128, H], F32, tag="lns", bufs=4, name="t_r1")
                nc.vector.tensor_add(t[:], attnF[:, sc * H:(sc + 1) * H], obt[:])
                r = sb.tile([128, H], F32, tag="lns", bufs=4, name="r_r1")
                nc.vector.tensor_add(r[:], t[:], x[:, sc * H:(sc + 1) * H])
                ln_natural(r[:], l1g[:], l1b[:], x1[:, sc * H:(sc + 1) * H])

            # --- x1^T ---
            x1T = sb.tile([128, NHC * S], F32R if USE_R else F32, tag="xT",
                          name="x1T")
            for hc in range(NHC):
                for sc in range(NSC):
                    transpose_128(
                        x1T[:, hc * S + sc * 128: hc * S + (sc + 1) * 128],
                        x1[:, sc * H + hc * 128: sc * H + (hc + 1) * 128])

            # --- W1 + gelu ---
            w1 = load_matrix(w1_d, li, H, FFHALF, 2)             # 3 tiles
            hT = sb.tile([128, NFFC * S], F32R if USE_R else F32, tag="hT",
                         name="hT")
            for m in range(NFFC):
                acc = ps.tile([128, S], F32, tag="mm", name="acc_h")
                for k in range(NHC):
                    nc.tensor.matmul(
                        acc[:], w1(k)[:, m * 128:(m + 1) * 128],
                        x1T[:, k * S:(k + 1) * S],
                        start=(k == 0), stop=(k == NHC - 1))
                nc.scalar.activation(
                    hT[:, m * S:(m + 1) * S], acc[:], AF.Gelu,
                    bias=qkb[:, 2 * NQK + m: 2 * NQK + m + 1])

            # --- W2 (partial) ---
            w2 = load_matrix(w2_d, li, FFHALF, H, 4)             # 3 tiles
            ffP = sb.tile([128, NSC * H], F32, tag="mmout", name="ffP")
            for sc in range(NSC):
                for n in range(2):
                    nw = 512 if n == 0 else H - 512
                    acc = ps.tile([128, S], F32, tag="mm", name="acc_f")
                    for k in range(NFFC):
                        nc.tensor.matmul(
                            acc[:, :nw],
                            hT[:, k * S + sc * 128: k * S + (sc + 1) * 128],
                            w2(k)[:, n * 512: n * 512 + nw],
                            start=(k == 0), stop=(k == NFFC - 1))
                    nc.scalar.copy(
                        ffP[:, sc * H + n * 512: sc * H + n * 512 + nw],
                        acc[:, :nw])

            # --- AllReduce #2 ---
            arin2 = dram.tile([S, H], F32, tag="arin", name="arin2")
            arout2 = dram.tile([S, H], F32, tag="arout", name="arout2")
            for sc in range(NSC):
                nc.sync.dma_start(arin2[sc * 128:(sc + 1) * 128, :],
                                  ffP[:, sc * H:(sc + 1) * H])
            if NO_AR:
                nc.sync.dma_start(arout2[:], arin2[:])
            else:
                nc.gpsimd.collective_compute(
                    "AllReduce", ALU.add, replica_groups=PAIRS,
                    ins=[arin2.opt()], outs=[arout2.opt()])
            ffF = sb.tile([128, NSC * H], F32, tag="mmout", name="ffF")
            for sc in range(NSC):
                nc.sync.dma_start(ffF[:, sc * H:(sc + 1) * H],
                                  arout2[sc * 128:(sc + 1) * 128, :])

            # --- residual + bias + LN2 ---
            b2t = param_tile(b2_d, li, "b2t")
            l2g = param_tile(ln2g_d, li, "l2g")
            l2b = param_tile(ln2b_d, li, "l2b")
            x_next = sb.tile([128, NSC * H], F32, tag="x", bufs=2, name="x_n")
            for sc in range(NSC):
                t = sb.tile([128, H], F32, tag="lns", bufs=4, name="t_r2")
                nc.vector.tensor_add(t[:], ffF[:, sc * H:(sc + 1) * H], b2t[:])
                r = sb.tile([128, H], F32, tag="lns", bufs=4, name="r_r2")
                nc.vector.tensor_add(r[:], t[:], x1[:, sc * H:(sc + 1) * H])
                ln_natural(r[:], l2g[:], l2b[:], x_next[:, sc * H:(sc + 1) * H])
            x = x_next

        # ---------------- final pairwise-distance block ----------------
        xTf = sb.tile([128, NHC * S], F32, tag="xT", name="xTf")
        for hc in range(NHC):
            for sc in range(NSC):
                transpose_128(
                    xTf[:, hc * S + sc * 128: hc * S + (sc + 1) * 128],
                    x[:, sc * H + hc * 128: sc * H + (hc + 1) * 128])

        # our query columns of x^T (half g, blended via hsel one-hot)
        xqT = sb.tile([128, NHC * SQ], F32, tag="qT", name="xqT")
        for hc in range(NHC):
            t0 = sb.tile([128, SQ], F32, tag="sm", bufs=3, name="t0sel")
            nc.vector.tensor_scalar_mul(t0[:], xTf[:, hc * S: hc * S + SQ],
                                        hsel[:, 0:1])
            nc.vector.scalar_tensor_tensor(
                xqT[:, hc * SQ:(hc + 1) * SQ],
                xTf[:, hc * S + SQ: hc * S + S], hsel[:, 1:2], t0[:],
                op0=ALU.mult, op1=ALU.add)

        # sq_k [1, S] then broadcast [128, S]
        sqk_ps = ps.tile([128, S], F32, tag="mm", name="sqk_ps")
        for hc in range(NHC):
            sqt = sb.tile([128, S], F32, tag="sm", bufs=3, name="sqt")
            nc.scalar.activation(sqt[:], xTf[:, hc * S:(hc + 1) * S], AF.Square)
            nc.tensor.matmul(sqk_ps[0:1, :], ones[:, 0:1], sqt[:],
                             start=(hc == 0), stop=(hc == NHC - 1))
        sqk = stat.tile([1, S], F32, tag="inv", bufs=2, name="sqk")
        nc.vector.tensor_copy(sqk[:], sqk_ps[0:1, :])
        bps_f = ps.tile([128, S], F32, tag="mm", name="bps_f")
        nc.tensor.matmul(bps_f[:], ones[0:1, :], sqk[:], start=True, stop=True)
        finb = sb.tile([128, NQK * S], F32, tag="kT", name="finb")
        sqk_bc = finb[:, 0:S]
        nc.vector.tensor_copy(sqk_bc, bps_f[:])

        # sq_q per q-chunk [128, 1]: square+accum on x chunks, blend by hsel
        sqx = []
        for sc in range(NSC):
            scr = sb.tile([128, H], F32, tag="lns", bufs=4, name="scr_sq")
            acc = stat.tile([128, 1], F32, tag="stat", name="sqx_acc")
            nc.scalar.activation(scr[:], x[:, sc * H:(sc + 1) * H], AF.Square,
                                 accum_out=acc[:])
            sqx.append(acc)
        sqq = []
        for qc in range(2):
            s0 = stat.tile([128, 1], F32, tag="stat", name="s0_sel")
            nc.vector.tensor_scalar_mul(s0[:], sqx[qc][:], hsel[:, 0:1])
            sq_ = stat.tile([128, 1], F32, tag="stat", name="sq_sel")
            nc.vector.scalar_tensor_tensor(sq_[:], sqx[2 + qc][:], hsel[:, 1:2],
                                           s0[:], op0=ALU.mult, op1=ALU.add)
            sqq.append(sq_)

        # op embeddings
        opt_t = sb.tile([128, NHC * NOPS], F32, tag="opT", name="opt_t")
        for hc in range(NHC):
            nc.sync.dma_start(opt_t[:, hc * NOPS:(hc + 1) * NOPS],
                              opT_d[hc * 128:(hc + 1) * 128, :])
        sqo_ps = ps.tile([128, S], F32, tag="mm", name="sqo_ps")
        for hc in range(NHC):
            sqt2 = sb.tile([128, NOPS], F32, tag="sqot", bufs=2, name="sqt2")
            nc.scalar.activation(sqt2[:], opt_t[:, hc * NOPS:(hc + 1) * NOPS],
                                 AF.Square)
            nc.tensor.matmul(sqo_ps[0:1, :NOPS], ones[:, 0:1], sqt2[:],
                             start=(hc == 0), stop=(hc == NHC - 1))
        sqo = stat.tile([1, NOPS], F32, tag="sqo", bufs=2, name="sqo")
        nc.vector.tensor_copy(sqo[:], sqo_ps[0:1, :NOPS])
        bps_o = ps.tile([128, S], F32, tag="mm", name="bps_o")
        nc.tensor.matmul(bps_o[:, :NOPS], ones[0:1, :], sqo[:],
                         start=True, stop=True)
        sqo_bc = sb.tile([128, NOPS], F32, tag="sqobc", name="sqo_bc")
        nc.vector.tensor_copy(sqo_bc[:], bps_o[:, :NOPS])

        dist_t, disto_t = [], []
        for qc in range(2):
            gps = ps.tile([128, S], F32, tag="mm", name="gps")
            for hc in range(NHC):
                nc.tensor.matmul(
                    gps[:], xqT[:, hc * SQ + qc * 128: hc * SQ + (qc + 1) * 128],
                    xTf[:, hc * S:(hc + 1) * S],
                    start=(hc == 0), stop=(hc == NHC - 1))
            d2 = sb.tile([128, S], F32, tag="sm", bufs=3, name="d2")
            nc.vector.scalar_tensor_tensor(d2[:], gps[:], -2.0, sqk_bc[:],
                                           op0=ALU.mult, op1=ALU.add)
            d2b = sb.tile([128, S], F32, tag="sm", bufs=3, name="d2b")
            nc.vector.tensor_scalar(d2b[:], d2[:], sqq[qc][:], 0.0,
                                    op0=ALU.add, op1=ALU.max)
            dt = finb[:, (1 + qc) * S:(2 + qc) * S]
            nc.scalar.activation(dt, d2b[:], AF.Sqrt)
            dist_t.append(dt)

            gops = ps.tile([128, S], F32, tag="mm", name="gops")
            for hc in range(NHC):
                nc.tensor.matmul(
                    gops[:, :NOPS],
                    xqT[:, hc * SQ + qc * 128: hc * SQ + (qc + 1) * 128],
                    opt_t[:, hc * NOPS:(hc + 1) * NOPS],
                    start=(hc == 0), stop=(hc == NHC - 1))
            d2o = sb.tile([128, NOPS], F32, tag="disto", bufs=6, name="d2o")
            nc.vector.scalar_tensor_tensor(d2o[:], gops[:, :NOPS], -2.0,
                                           sqo_bc[:], op0=ALU.mult, op1=ALU.add)
            d2ob = sb.tile([128, NOPS], F32, tag="disto", bufs=6, name="d2ob")
            nc.vector.tensor_scalar(d2ob[:], d2o[:], sqq[qc][:], 0.0,
                                    op0=ALU.add, op1=ALU.max)
            dto = sb.tile([128, NOPS], F32, tag="disto", bufs=6, name="dto")
            nc.scalar.activation(dto[:], d2ob[:], AF.Sqrt)
            disto_t.append(dto)

        # partial sums -> [1, 2] -> global AllReduce -> means
        r0 = stat.tile([128, 1], F32, tag="stat", name="r0")
        r1 = stat.tile([128, 1], F32, tag="stat", name="r1")
        rsum = stat.tile([128, 1], F32, tag="stat", name="rsum")
        nc.vector.tensor_reduce(r0[:], dist_t[0], axis=mybir.AxisListType.X,
                                op=ALU.add)
        nc.vector.tensor_reduce(r1[:], dist_t[1], axis=mybir.AxisListType.X,
                                op=ALU.add)
        nc.vector.tensor_add(rsum[:], r0[:], r1[:])
        ro0 = stat.tile([128, 1], F32, tag="stat", name="ro0")
        ro1 = stat.tile([128, 1], F32, tag="stat", name="ro1")
        rosum = stat.tile([128, 1], F32, tag="stat", name="rosum")
        nc.vector.tensor_reduce(ro0[:], disto_t[0][:],
                                axis=mybir.AxisListType.X, op=ALU.add)
        nc.vector.tensor_reduce(ro1[:], disto_t[1][:],
                                axis=mybir.AxisListType.X, op=ALU.add)
        nc.vector.tensor_add(rosum[:], ro0[:], ro1[:])
        tots = sb.tile([1, 2], F32, tag="tots", bufs=2, name="tots")
        tps_a = ps.tile([128, S], F32, tag="mm", name="tps_a")
        nc.tensor.matmul(tps_a[0:1, 0:1], ones[:, 0:1], rsum[:],
                         start=True, stop=True)
        nc.vector.tensor_copy(tots[:, 0:1], tps_a[0:1, 0:1])
        tps_b = ps.tile([128, S], F32, tag="mm", name="tps_b")
        nc.tensor.matmul(tps_b[0:1, 0:1], ones[:, 0:1], rosum[:],
                         start=True, stop=True)
        nc.vector.tensor_copy(tots[:, 1:2], tps_b[0:1, 0:1])

        arin3 = dram.tile([1, 2], F32, tag="arin3", name="arin3")
        arout3 = dram.tile([1, 2], F32, tag="arout3", name="arout3")
        nc.sync.dma_start(arin3[:], tots[:])
        nc.gpsimd.collective_compute(
            "AllReduce", ALU.add, replica_groups=[list(range(N_CORES))],
            ins=[arin3.opt()], outs=[arout3.opt()])
        ar3 = sb.tile([1, 2], F32, tag="tots", bufs=2, name="ar3")
        nc.sync.dma_start(ar3[:], arout3[:])
        means = stat.tile([1, 2], F32, tag="sqo", bufs=2, name="means")
        nc.vector.tensor_mul(means[:], ar3[:], scal[:])
        mb_ps = ps.tile([128, S], F32, tag="mm", name="mb_ps")
        nc.tensor.matmul(mb_ps[:, 0:2], ones[0:1, :], means[:],
                         start=True, stop=True)
        mean_bc = sb.tile([128, 2], F32, tag="meanbc", name="mean_bc")
        nc.vector.tensor_copy(mean_bc[:], mb_ps[:, 0:2])

        sigblk = sb.tile([128, NSC * H], F32, tag="mmout", name="sigblk")
        for qc in range(2):
            sig = sigblk[:, 0:S] if qc == 0 else sigblk[:, 3 * S:4 * S]
            nc.scalar.activation(sig, dist_t[qc], AF.Sigmoid,
                                 bias=mean_bc[:, 0:1], scale=-1.0)
            msk = sigblk[:, S:2 * S] if qc == 0 else sigblk[:, 4 * S:5 * S]
            nc.vector.tensor_scalar(msk, sig, THRESH, None, op0=ALU.is_ge)
            ww = sigblk[:, 2 * S:3 * S] if qc == 0 else sigblk[:, 5 * S:6 * S]
            nc.vector.tensor_mul(ww, sig, msk)
            nc.sync.dma_start(ww_out[qc * 128:(qc + 1) * 128, :], ww)

            sigo = sb.tile([128, NOPS], F32, tag="disto", bufs=6, name="sigo")
            nc.scalar.activation(sigo[:], disto_t[qc][:], AF.Sigmoid,
                                 bias=mean_bc[:, 1:2], scale=-1.0)
            msko = sb.tile([128, NOPS], F32, tag="disto", bufs=6, name="msko")
            nc.vector.tensor_scalar(msko[:], sigo[:], THRESH, None,
                                    op0=ALU.is_ge)
            wwo = sb.tile([128, NOPS], F32, tag="disto", bufs=6, name="wwo")
            nc.vector.tensor_mul(wwo[:], sigo[:], msko[:])
            nc.sync.dma_start(wop_out[qc * 128:(qc + 1) * 128, :], wwo[:])

        # embedded output (our half rows, chunk = 2*g + qc, blended by hsel)
        for qc in range(2):
            t0e = sb.tile([128, H], F32, tag="lns", bufs=4, name="t0e")
            nc.vector.tensor_scalar_mul(t0e[:], x[:, qc * H:(qc + 1) * H],
                                        hsel[:, 0:1])
            emb_sel = sb.tile([128, H], F32, tag="lns", bufs=4, name="emb_sel")
            nc.vector.scalar_tensor_tensor(
                emb_sel[:], x[:, (2 + qc) * H:(3 + qc) * H], hsel[:, 1:2],
                t0e[:], op0=ALU.mult, op1=ALU.add)
            nc.sync.dma_start(emb_out[qc * 128:(qc + 1) * 128, :], emb_sel[:])

    nc.finalize()
    return nc


# ---------------------------------------------------------------------------
# Host side: input prep, runner, output assembly
# ---------------------------------------------------------------------------

_RUNNER = None


class _Runner:
    def __init__(self):
        self.nc = build_program()
        nc = self.nc
        bass2jax.install_neuronx_cc_hook()
        partition_name = (nc.partition_id_tensor.name
                          if nc.partition_id_tensor else None)
        in_names, out_names, out_avals, zero_outs = [], [], [], []
        for alloc in nc.m.functions[0].allocations:
            if not isinstance(alloc, mybir.MemoryLocationSet):
                continue
            name = alloc.memorylocations[0].name
            if alloc.kind == "ExternalInput":
                if name != partition_name:
                    in_names.append(name)
            elif alloc.kind == "ExternalOutput":
                out_names.append(name)
                shape = tuple(alloc.tensor_shape)
                dtype = mybir.dt.np(alloc.dtype)
                out_avals.append(jax.core.ShapedArray(shape, dtype))
                zero_outs.append(np.zeros(shape, dtype))
        self.n_params = len(in_names)
        self.param_names = list(in_names)
        self.out_names = out_names
        self.out_avals = out_avals
        self.zero_outs = zero_outs
        all_in = in_names + out_names
        if partition_name:
            all_in.append(partition_name)

        def _body(*args):
            operands = list(args)
            if partition_name:
                operands.append(bass2jax.partition_id_tensor())
            outs = bass2jax._bass_exec_p.bind(
                *operands, out_avals=tuple(out_avals),
                in_names=tuple(all_in), out_names=tuple(out_names),
                lowering_input_output_aliases=(), sim_require_finite=True,
                sim_require_nnan=True, nc=nc)
            return tuple(outs)

        devices = jax.devices()[:N_CORES]
        self.mesh = Mesh(np.asarray(devices), ("core",))
        n_in = self.n_params + len(out_names)
        self.sharded = jax.jit(
            shard_map(_body, mesh=self.mesh,
                      in_specs=(PartitionSpec("core"),) * n_in,
                      out_specs=(PartitionSpec("core"),) * len(out_names),
                      check_rep=False),
            keep_unused=True)
        self.sharding = NamedSharding(self.mesh, PartitionSpec("core"))

    def place(self, per_core_maps):
        args = []
        for name in self.param_names:
            cat = np.concatenate([np.asarray(m[name]) for m in per_core_maps],
                                 axis=0)
            args.append(jax.device_put(cat, self.sharding))
        for z in self.zero_outs:
            cat = np.zeros((N_CORES * z.shape[0], *z.shape[1:]), z.dtype)
            args.append(jax.device_put(cat, self.sharding))
        return args

    def run_raw(self, args):
        return self.sharded(*args)

    def run(self, args):
        outs = self.sharded(*args)
        jax.block_until_ready(outs)
        return {name: np.asarray(o).reshape(N_CORES, *self.out_avals[i].shape)
                for i, (name, o) in enumerate(
                    zip(self.out_names, outs))}


def get_runner():
    global _RUNNER
    if _RUNNER is None:
        _RUNNER = _Runner()
    return _RUNNER


def prepare_core_inputs(inputs, core):
    p, g = core // 2, core % 2
    f32 = np.float32
    ids = np.asarray(inputs["input_ids"])[p]
    x0 = (np.asarray(inputs["word_emb"], f32)[ids]
          + np.asarray(inputs["pos_emb"], f32)[:S]
          + np.asarray(inputs["type_emb"], f32)[0][None, :]).astype(f32)
    am = np.asarray(inputs["attention_mask"])[p].astype(f32)
    amask = np.ascontiguousarray(
        ((1.0 - am) * -1e9).astype(f32).reshape(NSC, 128).T)

    rep = lambda v: np.repeat(np.asarray(v, f32).reshape(1, -1), 128, axis=0)
    embg = rep(inputs["emb_ln_g"])
    embb = rep(inputs["emb_ln_b"])

    Wqkv = np.asarray(inputs["Wqkv"], f32)
    bqkv = np.asarray(inputs["bqkv"], f32)
    Wo = np.asarray(inputs["Wo"], f32)
    bo = np.asarray(inputs["bo"], f32)
    W1 = np.asarray(inputs["W1"], f32)
    b1 = np.asarray(inputs["b1"], f32)
    W2 = np.asarray(inputs["W2"], f32)
    b2 = np.asarray(inputs["b2"], f32)

    qs = slice(g * HHALF, (g + 1) * HHALF)
    ks = slice(H + g * HHALF, H + (g + 1) * HHALF)
    vs = slice(2 * H + g * HHALF, 2 * H + (g + 1) * HHALF)
    wqkv = np.ascontiguousarray(
        np.concatenate([Wqkv[:, :, qs], Wqkv[:, :, ks], Wqkv[:, :, vs]],
                       axis=2))[:NL]
    wo = np.ascontiguousarray(Wo[:, g * HHALF:(g + 1) * HHALF, :])[:NL]
    w1 = np.ascontiguousarray(W1[:, :, g * FFHALF:(g + 1) * FFHALF])[:NL]
    w2 = np.ascontiguousarray(W2[:, g * FFHALF:(g + 1) * FFHALF, :])[:NL]

    nl = NL
    qkb = np.zeros((nl, 128, 2 * NQK + NFFC), f32)
    for m in range(NQK):
        qkb[:, :, m] = bqkv[:nl, g * HHALF + m * 128: g * HHALF + (m + 1) * 128]
        qkb[:, :, NQK + m] = bqkv[:nl, H + g * HHALF + m * 128:
                                  H + g * HHALF + (m + 1) * 128]
    for m in range(NFFC):
        qkb[:, :, 2 * NQK + m] = b1[:nl, g * FFHALF + m * 128:
                                    g * FFHALF + (m + 1) * 128]
    vb = np.repeat(bqkv[:nl, None, 2 * H + g * HHALF: 2 * H + (g + 1) * HHALF],
                   128, axis=1).astype(f32)
    repl = lambda a: np.repeat(np.asarray(a, f32)[:nl, None, :], 128, axis=1)
    ob = repl(bo)
    b2r = repl(b2)
    ln1g = repl(inputs["ln1_g"])
    ln1b = repl(inputs["ln1_b"])
    ln2g = repl(inputs["ln2_g"])
    ln2b = repl(inputs["ln2_b"])

    opT = np.ascontiguousarray(np.asarray(inputs["op_emb"], f32).T)
    scal = np.array([[1.0 / (B * S * S), 1.0 / (B * S * NOPS)]], f32)
    hsel = np.zeros((128, 2), f32)
    hsel[:, g] = 1.0

    return {
        "x0": x0, "amask": amask, "embg": embg, "embb": embb,
        "wqkv": wqkv, "wo": wo, "w1": w1, "w2": w2,
        "qkb": qkb, "vb": vb, "ob": ob, "b2": b2r,
        "ln1g": ln1g, "ln1b": ln1b, "ln2g": ln2g, "ln2b": ln2b,
        "opT": opT, "scal": scal, "hsel": hsel,
    }


def assemble_outputs(res):
    f32 = np.float32
    embedded = np.zeros((B, S, H), f32)
    word_word = np.zeros((B, S, S), f32)
    word_operator = np.zeros((B, S, NOPS), f32)
    for c in range(N_CORES):
        p, g = c // 2, c % 2
        rows = slice(g * SQ, (g + 1) * SQ)
        embedded[p, rows] = res["emb_out"][c]
        word_word[p, rows] = res["ww_out"][c]
        word_operator[p, rows] = res["wop_out"][c]
    return embedded, word_word, word_operator


def kernel(**inputs):
    r = get_runner()
    maps = [prepare_core_inputs(inputs, c) for c in range(N_CORES)]
    args = r.place(maps)
    res = r.run(args)
    return assemble_outputs(res)


# revision 11
# speedup vs baseline: 2.9628x; 2.8710x over previous
"""Trainium2 Bass kernel for nn_BertEncoder_45260365365543.

BERT-base encoder (12 layers, B=4, S=512, H=768) + pairwise L2-distance
outputs, on 8 NeuronCores:
  - 4 pairs of cores; pair p handles batch element p.
  - Within a pair: tensor-parallel split (6 of 12 heads, half of the FFN
    per core) with two pairwise AllReduces per layer (after the Wo partial
    and after the W2 partial).
  - Final pairwise-distance block: both cores of a pair hold the full
    embedded sequence after the last AllReduce; each computes its half of
    the query rows of word_word / word_operator locally. One global
    AllReduce produces the distance means.

All compute is fp32 (PE fp32 matmuls, fp32 PSUM accumulation) to keep the
sigmoid-threshold outputs numerically faithful to the fp32 reference.
"""

import os
import sys

sys.path.insert(0, "/opt/trn_rl_repo")

import numpy as np

import concourse.bass as bass
import concourse.mybir as mybir
import concourse.tile as tile
from concourse import bacc, bass2jax
from concourse.masks import make_identity

import jax
from jax.sharding import Mesh, PartitionSpec, NamedSharding
from jax.experimental.shard_map import shard_map

F32 = mybir.dt.float32
AF = mybir.ActivationFunctionType
ALU = mybir.AluOpType

NL = int(os.environ.get("BK_NL", "12"))
USE_R = os.environ.get("BK_R", "0") == "1"
NO_AR = os.environ.get("BK_NOAR", "0") == "1"   # timing ablation only
F32R = mybir.dt.float32r
H, NH, DH, FF, S, B = 768, 12, 64, 3072, 512, 4
NOPS = 16
THRESH = 0.4
EPS = 1e-12
SCALE = 1.0 / float(np.sqrt(DH))

HHALF = H // 2            # 384 features (6 heads) per core
FFHALF = FF // 2          # 1536
NSC = S // 128            # 4 sequence chunks
NHC = H // 128            # 6 feature chunks
NFFC = FFHALF // 128      # 12 ff chunks per core
NQK = HHALF // 128        # 3 chunks for Q^T / K^T halves
NHEADS = NH // 2          # 6 heads per core
SQ = S // 2               # query rows handled per core

N_CORES = 8
PAIRS = [[2 * p, 2 * p + 1] for p in range(4)]


def build_program():
    nc = bacc.Bacc()

    # ---------------- DRAM I/O ----------------
    x0_d = nc.dram_tensor("x0", [S, H], F32, kind="ExternalInput")
    amask_d = nc.dram_tensor("amask", [128, NSC], F32, kind="ExternalInput")
    embg_d = nc.dram_tensor("embg", [128, H], F32, kind="ExternalInput")
    embb_d = nc.dram_tensor("embb", [128, H], F32, kind="ExternalInput")
    wqkv_d = nc.dram_tensor("wqkv", [NL, H, 3 * HHALF], F32, kind="ExternalInput")
    wo_d = nc.dram_tensor("wo", [NL, HHALF, H], F32, kind="ExternalInput")
    w1_d = nc.dram_tensor("w1", [NL, H, FFHALF], F32, kind="ExternalInput")
    w2_d = nc.dram_tensor("w2", [NL, FFHALF, H], F32, kind="ExternalInput")
    qkb_d = nc.dram_tensor("qkb", [NL, 128, 2 * NQK + NFFC], F32,
                           kind="ExternalInput")
    vb_d = nc.dram_tensor("vb", [NL, 128, HHALF], F32, kind="ExternalInput")
    ob_d = nc.dram_tensor("ob", [NL, 128, H], F32, kind="ExternalInput")
    b2_d = nc.dram_tensor("b2", [NL, 128, H], F32, kind="ExternalInput")
    ln1g_d = nc.dram_tensor("ln1g", [NL, 128, H], F32, kind="ExternalInput")
    ln1b_d = nc.dram_tensor("ln1b", [NL, 128, H], F32, kind="ExternalInput")
    ln2g_d = nc.dram_tensor("ln2g", [NL, 128, H], F32, kind="ExternalInput")
    ln2b_d = nc.dram_tensor("ln2b", [NL, 128, H], F32, kind="ExternalInput")
    opT_d = nc.dram_tensor("opT", [H, NOPS], F32, kind="ExternalInput")
    scal_d = nc.dram_tensor("scal", [1, 2], F32, kind="ExternalInput")
    hsel_d = nc.dram_tensor("hsel", [128, 2], F32, kind="ExternalInput")

    emb_out = nc.dram_tensor("emb_out", [SQ, H], F32, kind="ExternalOutput")
    ww_out = nc.dram_tensor("ww_out", [SQ, S], F32, kind="ExternalOutput")
    wop_out = nc.dram_tensor("wop_out", [SQ, NOPS], F32, kind="ExternalOutput")

    from contextlib import ExitStack
    with tile.TileContext(nc) as tc, ExitStack() as ctx:
        const = ctx.enter_context(tc.tile_pool(name="const", bufs=1))
        sb = ctx.enter_context(tc.tile_pool(name="sb", bufs=1))
        stat = ctx.enter_context(tc.tile_pool(name="stat", bufs=12))
        wpool = ctx.enter_context(tc.tile_pool(name="wpool", bufs=4))
        ps = ctx.enter_context(tc.tile_pool(name="ps", bufs=4, space="PSUM"))
        ps2 = ctx.enter_context(tc.tile_pool(name="ps2", bufs=2, space="PSUM"))
        dram = ctx.enter_context(tc.tile_pool(name="dram", bufs=1, space="DRAM"))

        ones = const.tile([128, 128], F32)
        nc.gpsimd.memset(ones[:], 1.0)
        ident = const.tile([128, 128], F32)
        make_identity(nc, ident[:])
        amask = const.tile([128, NSC], F32)
        nc.sync.dma_start(amask[:], amask_d[:])
        scal = const.tile([1, 2], F32)
        nc.sync.dma_start(scal[:], scal_d[:])
        hsel = const.tile([128, 2], F32)
        nc.sync.dma_start(hsel[:], hsel_d[:])
        epst = const.tile([128, 1], F32)
        nc.gpsimd.memset(epst[:], EPS)

        def transpose_128(out_slice, in_slice):
            tp = ps2.tile([128, 128], F32, tag="tp", name="tp")
            nc.tensor.transpose(tp[:], in_slice, ident[:])
            nc.vector.tensor_copy(out_slice, tp[:])

        def ln_natural(x_slice, g_tile, b_tile, out_slice):
            """LayerNorm along the free dim of a [128, H] slice."""
            mus = stat.tile([128, 1], F32, tag="stat", name="mus")
            nc.vector.tensor_reduce(mus[:], x_slice, axis=mybir.AxisListType.X,
                                    op=ALU.add)
            mu = stat.tile([128, 1], F32, tag="stat", name="mu")
            nc.scalar.mul(mu[:], mus[:], 1.0 / H)
            xc = sb.tile([128, H], F32, tag="lns", bufs=4, name="xc")
            nc.vector.tensor_scalar_sub(xc[:], x_slice, mu[:])
            sq = sb.tile([128, H], F32, tag="lns", bufs=4, name="sq")
            ss = stat.tile([128, 1], F32, tag="stat", name="ss")
            nc.scalar.activation(sq[:], xc[:], AF.Square, accum_out=ss[:])
            std = stat.tile([128, 1], F32, tag="stat", name="std")
            nc.scalar.activation(std[:], ss[:], AF.Sqrt, bias=epst[:], scale=1.0 / H)
            istd = stat.tile([128, 1], F32, tag="stat", name="istd")
            nc.vector.reciprocal(istd[:], std[:])
            t2 = sb.tile([128, H], F32, tag="lns", bufs=4, name="t2")
            nc.vector.scalar_tensor_tensor(t2[:], xc[:], istd[:], g_tile,
                                           op0=ALU.mult, op1=ALU.mult)
            nc.vector.tensor_add(out_slice, t2[:], b_tile)

        def load_matrix(dram3, li, nrows, ncols, slabs_per_tile):
            """Load [nrows, ncols] matrix (layer li) into tiles of
            slabs_per_tile row-slabs each; returns slab accessor."""
            nslab = nrows // 128
            tiles = []
            for t0 in range((nslab + slabs_per_tile - 1) // slabs_per_tile):
                cnt = min(slabs_per_tile, nslab - t0 * slabs_per_tile)
                wdt = F32R if USE_R else F32
                wt = wpool.tile([128, cnt * ncols], wdt, tag="wb", bufs=3,
                                name="wt")
                for j in range(cnt):
                    k = t0 * slabs_per_tile + j
                    eng = nc.gpsimd if USE_R else nc.sync
                    eng.dma_start(wt[:, j * ncols:(j + 1) * ncols],
                                  dram3[li, k * 128:(k + 1) * 128, :])
                tiles.append(wt)
            return lambda k: tiles[k // slabs_per_tile][
                :, (k % slabs_per_tile) * ncols:((k % slabs_per_tile) + 1) * ncols]

        def param_tile(dram3, li, name):
            t = sb.tile([128, H], F32, tag="param", bufs=4, name=name)
            nc.sync.dma_start(t[:], dram3[li])
            return t

        # ---------------- embeddings + LN ----------------
        x = sb.tile([128, NSC * H], F32, tag="x", bufs=2, name="x_emb")
        x0t = sb.tile([128, NSC * H], F32, tag="mmout", name="x0t")
        for sc in range(NSC):
            nc.sync.dma_start(x0t[:, sc * H:(sc + 1) * H],
                              x0_d[sc * 128:(sc + 1) * 128, :])
        embg = sb.tile([128, H], F32, tag="param", bufs=4, name="embg")
        nc.sync.dma_start(embg[:], embg_d[:])
        embb = sb.tile([128, H], F32, tag="param", bufs=4, name="embb")
        nc.sync.dma_start(embb[:], embb_d[:])
        for sc in range(NSC):
            ln_natural(x0t[:, sc * H:(sc + 1) * H], embg[:], embb[:],
                       x[:, sc * H:(sc + 1) * H])

        # ---------------- encoder layers ----------------
        for li in range(NL):
            wq = load_matrix(wqkv_d, li, H, 3 * HHALF, 2)       # 3 tiles
            qkb = sb.tile([128, 2 * NQK + NFFC], F32, tag="qkb", bufs=2,
                          name="qkb")
            nc.sync.dma_start(qkb[:], qkb_d[li])
            vb = sb.tile([128, HHALF], F32, tag="vb", bufs=2, name="vb")
            nc.sync.dma_start(vb[:], vb_d[li])

            xT = sb.tile([128, NHC * S], F32R if USE_R else F32, tag="xT",
                         name="xT")
            for hc in range(NHC):
                for sc in range(NSC):
                    transpose_128(
                        xT[:, hc * S + sc * 128: hc * S + (sc + 1) * 128],
                        x[:, sc * H + hc * 128: sc * H + (hc + 1) * 128])

            # --- QKV ---
            qT = sb.tile([128, NQK * S], F32, tag="qT", name="qT")
            kT = sb.tile([128, NQK * S], F32, tag="kT", name="kT")
            for qk in range(2):
                dst = qT if qk == 0 else kT
                for m in range(NQK):
                    acc = ps.tile([128, S], F32, tag="mm", name="acc_qk")
                    for k in range(NHC):
                        nc.tensor.matmul(
                            acc[:],
                            wq(k)[:, qk * HHALF + m * 128: qk * HHALF + (m + 1) * 128],
                            xT[:, k * S:(k + 1) * S],
                            start=(k == 0), stop=(k == NHC - 1))
                    nc.scalar.activation(dst[:, m * S:(m + 1) * S], acc[:],
                                         AF.Identity,
                                         bias=qkb[:, qk * NQK + m: qk * NQK + m + 1])
            vN = sb.tile([128, NSC * HHALF], F32, tag="vN", name="vN")
            for sc in range(NSC):
                acc = ps.tile([128, S], F32, tag="mm", name="acc_v")
                for k in range(NHC):
                    nc.tensor.matmul(
                        acc[:, :HHALF],
                        xT[:, k * S + sc * 128: k * S + (sc + 1) * 128],
                        wq(k)[:, 2 * HHALF:],
                        start=(k == 0), stop=(k == NHC - 1))
                nc.vector.tensor_add(vN[:, sc * HHALF:(sc + 1) * HHALF],
                                     acc[:, :HHALF], vb[:])

            # --- attention (6 heads) ---
            ctxT = sb.tile([128, NQK * S], F32R if USE_R else F32,
                           tag="ctxT", name="ctxT")
            for h in range(NHEADS):
                mt = h // 2
                ro = (h % 2) * 64
                qTh = qT[ro:ro + 64, mt * S:(mt + 1) * S]
                expt = sb.tile([128, NSC * S], F32, tag="exp", bufs=2,
                               name="expt")
                for kc in range(NSC):
                    sps = ps.tile([128, S], F32, tag="mm", name="sps")
                    nc.tensor.matmul(
                        sps[:],
                        kT[ro:ro + 64,
                           mt * S + kc * 128: mt * S + (kc + 1) * 128],
                        qTh, start=True, stop=True)
                    nc.scalar.activation(expt[:, kc * S:(kc + 1) * S], sps[:],
                                         AF.Exp, bias=amask[:, kc:kc + 1],
                                         scale=SCALE)
                d01 = sb.tile([128, S], F32, tag="sm", bufs=3, name="d01")
                d23 = sb.tile([128, S], F32, tag="sm", bufs=3, name="d23")
                dall = sb.tile([128, S], F32, tag="sm", bufs=3, name="dall")
                nc.vector.tensor_add(d01[:], expt[:, 0:S], expt[:, S:2 * S])
                nc.vector.tensor_add(d23[:], expt[:, 2 * S:3 * S],
                                     expt[:, 3 * S:4 * S])
                nc.vector.tensor_add(dall[:], d01[:], d23[:])
                dps = ps.tile([128, S], F32, tag="mm", name="dps")
                nc.tensor.matmul(dps[0:1, :], ones[:, 0:1], dall[:],
                                 start=True, stop=True)
                inv = stat.tile([1, S], F32, tag="inv", bufs=2, name="inv")
                nc.vector.reciprocal(inv[:], dps[0:1, :])
                bps = ps.tile([128, S], F32, tag="mm", name="bps")
                nc.tensor.matmul(bps[0:64, :], ones[0:1, 0:64], inv[:],
                                 start=True, stop=True)
                bcs = sb.tile([64, S], F32, tag="bcs", bufs=2, name="bcs")
                nc.scalar.copy(bcs[:], bps[0:64, :])
                cps = ps2.tile([64, S], F32, tag="ctx", name="cps")
                for kc in range(NSC):
                    nc.tensor.matmul(
                        cps[:],
                        vN[:, kc * HHALF + h * 64: kc * HHALF + (h + 1) * 64],
                        expt[:, kc * S:(kc + 1) * S],
                        start=(kc == 0), stop=(kc == NSC - 1))
                nc.vector.tensor_mul(
                    ctxT[ro:ro + 64, mt * S:(mt + 1) * S], cps[:], bcs[:])

            # --- Wo (partial) ---
            wo = load_matrix(wo_d, li, HHALF, H, 2)              # 2 tiles
            attnP = sb.tile([128, NSC * H], F32, tag="mmout", name="attnP")
            for sc in range(NSC):
                for n in range(2):
                    nw = 512 if n == 0 else H - 512
                    acc = ps.tile([128, S], F32, tag="mm", name="acc_o")
                    for k in range(NQK):
                        nc.tensor.matmul(
                            acc[:, :nw],
                            ctxT[:, k * S + sc * 128: k * S + (sc + 1) * 128],
                            wo(k)[:, n * 512: n * 512 + nw],
                            start=(k == 0), stop=(k == NQK - 1))
                    nc.scalar.copy(
                        attnP[:, sc * H + n * 512: sc * H + n * 512 + nw],
                        acc[:, :nw])

            # --- AllReduce #1 ---
            arin1 = dram.tile([S, H], F32, tag="arin", name="arin1")
            arout1 = dram.tile([S, H], F32, tag="arout", name="arout1")
            attnF = sb.tile([128, NSC * H], F32, tag="mmout", name="attnF")
            for hf in range(2):
                for sc in (2 * hf, 2 * hf + 1):
                    nc.sync.dma_start(arin1[sc * 128:(sc + 1) * 128, :],
                                      attnP[:, sc * H:(sc + 1) * H])
                nc.gpsimd.collective_compute(
                    "AllReduce", ALU.add, replica_groups=PAIRS,
                    ins=[arin1[hf * 256:(hf + 1) * 256, :].opt()],
                    outs=[arout1[hf * 256:(hf + 1) * 256, :].opt()])
                for sc in (2 * hf, 2 * hf + 1):
                    nc.sync.dma_start(attnF[:, sc * H:(sc + 1) * H],
                                      arout1[sc * 128:(sc + 1) * 128, :])

            # --- residual + bias + LN1 ---
            obt = param_tile(ob_d, li, "obt")
            l1g = param_tile(ln1g_d, li, "l1g")
            l1b = param_tile(ln1b_d, li, "l1b")
            x1 = sb.tile([128, NSC * H], F32, tag="x", bufs=2, name="x1")
            for sc in range(NSC):
                t = sb.tile([128, H], F32, tag="lns", bufs=4, name="t_r1")
                nc.vector.tensor_add(t[:], attnF[:, sc * H:(sc + 1) * H], obt[:])
                r = sb.tile([128, H], F32, tag="lns", bufs=4, name="r_r1")
                nc.vector.tensor_add(r[:], t[:], x[:, sc * H:(sc + 1) * H])
                ln_natural(r[:], l1g[:], l1b[:], x1[:, sc * H:(sc + 1) * H])

            # --- x1^T ---
            x1T = sb.tile([128, NHC * S], F32R if USE_R else F32, tag="xT",
                          name="x1T")
            for hc in range(NHC):
                for sc in range(NSC):
                    transpose_128(
                        x1T[:, hc * S + sc * 128: hc * S + (sc + 1) * 128],
                        x1[:, sc * H + hc * 128: sc * H + (hc + 1) * 128])

            # --- W1 + gelu ---
            w1 = load_matrix(w1_d, li, H, FFHALF, 2)             # 3 tiles
            hT = sb.tile([128, NFFC * S], F32R if USE_R else F32, tag="hT",
                         name="hT")
            for m in range(NFFC):
                acc = ps.tile([128, S], F32, tag="mm", name="acc_h")
                for k in range(NHC):
                    nc.tensor.matmul(
                        acc[:], w1(k)[:, m * 128:(m + 1) * 128],
                        x1T[:, k * S:(k + 1) * S],
                        start=(k == 0), stop=(k == NHC - 1))
                nc.scalar.activation(
                    hT[:, m * S:(m + 1) * S], acc[:], AF.Gelu,
                    bias=qkb[:, 2 * NQK + m: 2 * NQK + m + 1])

            # --- W2 (partial) ---
            w2 = load_matrix(w2_d, li, FFHALF, H, 4)             # 3 tiles
            ffP = sb.tile([128, NSC * H], F32, tag="mmout", name="ffP")
            for sc in range(NSC):
                for n in range(2):
                    nw = 512 if n == 0 else H - 512
                    acc = ps.tile([128, S], F32, tag="mm", name="acc_f")
                    for k in range(NFFC):
                        nc.tensor.matmul(
                            acc[:, :nw],
                            hT[:, k * S + sc * 128: k * S + (sc + 1) * 128],
                            w2(k)[:, n * 512: n * 512 + nw],
                            start=(k == 0), stop=(k == NFFC - 1))
                    nc.scalar.copy(
                        ffP[:, sc * H + n * 512: sc * H + n * 512 + nw],
                        acc[:, :nw])

            # --- AllReduce #2 ---
            arin2 = dram.tile([S, H], F32, tag="arin", name="arin2")
            arout2 = dram.tile([S, H], F32, tag="arout", name="arout2")
            ffF = sb.tile([128, NSC * H], F32, tag="mmout", name="ffF")
            for hf in range(2):
                for sc in (2 * hf, 2 * hf + 1):
                    nc.sync.dma_start(arin2[sc * 128:(sc + 1) * 128, :],
                                      ffP[:, sc * H:(sc + 1) * H])
                nc.gpsimd.collective_compute(
                    "AllReduce", ALU.add, replica_groups=PAIRS,
                    ins=[arin2[hf * 256:(hf + 1) * 256, :].opt()],
                    outs=[arout2[hf * 256:(hf + 1) * 256, :].opt()])
                for sc in (2 * hf, 2 * hf + 1):
                    nc.sync.dma_start(ffF[:, sc * H:(sc + 1) * H],
                                      arout2[sc * 128:(sc + 1) * 128, :])

            # --- residual + bias + LN2 ---
            b2t = param_tile(b2_d, li, "b2t")
            l2g = param_tile(ln2g_d, li, "l2g")
            l2b = param_tile(ln2b_d, li, "l2b")
            x_next = sb.tile([128, NSC * H], F32, tag="x", bufs=2, name="x_n")
            for sc in range(NSC):
                t = sb.tile([128, H], F32, tag="lns", bufs=4, name="t_r2")
                nc.vector.tensor_add(t[:], ffF[:, sc * H:(sc + 1) * H], b2t[:])
                r = sb.tile([128, H], F32, tag="lns", bufs=4, name="r_r2")
                nc.vector.tensor_add(r[:], t[:], x1[:, sc * H:(sc + 1) * H])
                ln_natural(r[:], l2g[:], l2b[:], x_next[:, sc * H:(sc + 1) * H])
            x = x_next

        # ---------------- final pairwise-distance block ----------------
        xTf = sb.tile([128, NHC * S], F32, tag="xT", name="xTf")
        for hc in range(NHC):
            for sc in range(NSC):
                transpose_128(
                    xTf[:, hc * S + sc * 128: hc * S + (sc + 1) * 128],
                    x[:, sc * H + hc * 128: sc * H + (hc + 1) * 128])

        # our query columns of x^T (half g, blended via hsel one-hot)
        xqT = sb.tile([128, NHC * SQ], F32, tag="qT", name="xqT")
        for hc in range(NHC):
            t0 = sb.tile([128, SQ], F32, tag="sm", bufs=3, name="t0sel")
            nc.vector.tensor_scalar_mul(t0[:], xTf[:, hc * S: hc * S + SQ],
                                        hsel[:, 0:1])
            nc.vector.scalar_tensor_tensor(
                xqT[:, hc * SQ:(hc + 1) * SQ],
                xTf[:, hc * S + SQ: hc * S + S], hsel[:, 1:2], t0[:],
                op0=ALU.mult, op1=ALU.add)

        # sq_k [1, S] then broadcast [128, S]
        sqk_ps = ps.tile([128, S], F32, tag="mm", name="sqk_ps")
        for hc in range(NHC):
            sqt = sb.tile([128, S], F32, tag="sm", bufs=3, name="sqt")
            nc.scalar.activation(sqt[:], xTf[:, hc * S:(hc + 1) * S], AF.Square)
            nc.tensor.matmul(sqk_ps[0:1, :], ones[:, 0:1], sqt[:],
                             start=(hc == 0), stop=(hc == NHC - 1))
        sqk = stat.tile([1, S], F32, tag="inv", bufs=2, name="sqk")
        nc.vector.tensor_copy(sqk[:], sqk_ps[0:1, :])
        bps_f = ps.tile([128, S], F32, tag="mm", name="bps_f")
        nc.tensor.matmul(bps_f[:], ones[0:1, :], sqk[:], start=True, stop=True)
        finb = sb.tile([128, NQK * S], F32, tag="kT", name="finb")
        sqk_bc = finb[:, 0:S]
        nc.vector.tensor_copy(sqk_bc, bps_f[:])

        # sq_q per q-chunk [128, 1]: square+accum on x chunks, blend by hsel
        sqx = []
        for sc in range(NSC):
            scr = sb.tile([128, H], F32, tag="lns", bufs=4, name="scr_sq")
            acc = stat.tile([128, 1], F32, tag="stat", name="sqx_acc")
            nc.scalar.activation(scr[:], x[:, sc * H:(sc + 1) * H], AF.Square,
                                 accum_out=acc[:])
            sqx.append(acc)
        sqq = []
        for qc in range(2):
            s0 = stat.tile([128, 1], F32, tag="stat", name="s0_sel")
            nc.vector.tensor_scalar_mul(s0[:], sqx[qc][:], hsel[:, 0:1])
            sq_ = stat.tile([128, 1], F32, tag="stat", name="sq_sel")
            nc.vector.scalar_tensor_tensor(sq_[:], sqx[2 + qc][:], hsel[:, 1:2],
                                           s0[:], op0=ALU.mult, op1=ALU.add)
            sqq.append(sq_)

        # op embeddings
        opt_t = sb.tile([128, NHC * NOPS], F32, tag="opT", name="opt_t")
        for hc in range(NHC):
            nc.sync.dma_start(opt_t[:, hc * NOPS:(hc + 1) * NOPS],
                              opT_d[hc * 128:(hc + 1) * 128, :])
        sqo_ps = ps.tile([128, S], F32, tag="mm", name="sqo_ps")
        for hc in range(NHC):
            sqt2 = sb.tile([128, NOPS], F32, tag="sqot", bufs=2, name="sqt2")
            nc.scalar.activation(sqt2[:], opt_t[:, hc * NOPS:(hc + 1) * NOPS],
                                 AF.Square)
            nc.tensor.matmul(sqo_ps[0:1, :NOPS], ones[:, 0:1], sqt2[:],
                             start=(hc == 0), stop=(hc == NHC - 1))
        sqo = stat.tile([1, NOPS], F32, tag="sqo", bufs=2, name="sqo")
        nc.vector.tensor_copy(sqo[:], sqo_ps[0:1, :NOPS])
        bps_o = ps.tile([128, S], F32, tag="mm", name="bps_o")
        nc.tensor.matmul(bps_o[:, :NOPS], ones[0:1, :], sqo[:],
                         start=True, stop=True)
        sqo_bc = sb.tile([128, NOPS], F32, tag="sqobc", name="sqo_bc")
        nc.vector.tensor_copy(sqo_bc[:], bps_o[:, :NOPS])

        dist_t, disto_t = [], []
        for qc in range(2):
            gps = ps.tile([128, S], F32, tag="mm", name="gps")
            for hc in range(NHC):
                nc.tensor.matmul(
                    gps[:], xqT[:, hc * SQ + qc * 128: hc * SQ + (qc + 1) * 128],
                    xTf[:, hc * S:(hc + 1) * S],
                    start=(hc == 0), stop=(hc == NHC - 1))
            d2 = sb.tile([128, S], F32, tag="sm", bufs=3, name="d2")
            nc.vector.scalar_tensor_tensor(d2[:], gps[:], -2.0, sqk_bc[:],
                                           op0=ALU.mult, op1=ALU.add)
            d2b = sb.tile([128, S], F32, tag="sm", bufs=3, name="d2b")
            nc.vector.tensor_scalar(d2b[:], d2[:], sqq[qc][:], 0.0,
                                    op0=ALU.add, op1=ALU.max)
            dt = finb[:, (1 + qc) * S:(2 + qc) * S]
            nc.scalar.activation(dt, d2b[:], AF.Sqrt)
            dist_t.append(dt)

            gops = ps.tile([128, S], F32, tag="mm", name="gops")
            for hc in range(NHC):
                nc.tensor.matmul(
                    gops[:, :NOPS],
                    xqT[:, hc * SQ + qc * 128: hc * SQ + (qc + 1) * 128],
                    opt_t[:, hc * NOPS:(hc + 1) * NOPS],
                    start=(hc == 0), stop=(hc == NHC - 1))
            d2o = sb.tile([128, NOPS], F32, tag="disto", bufs=6, name="d2o")
            nc.vector.scalar_tensor_tensor(d2o[:], gops[:, :NOPS], -2.0,
                                           sqo_bc[:], op0=ALU.mult, op1=ALU.add)
            d2ob = sb.tile([128, NOPS], F32, tag="disto", bufs=6, name="d2ob")
            nc.vector.tensor_scalar(d2ob[:], d2o[:], sqq[qc][:], 0.0,
                                    op0=ALU.add, op1=ALU.max)
            dto = sb.tile([128, NOPS], F32, tag="disto", bufs=6, name="dto")
            nc.scalar.activation(dto[:], d2ob[:], AF.Sqrt)
            disto_t.append(dto)

        # partial sums -> [1, 2] -> global AllReduce -> means
        r0 = stat.tile([128, 1], F32, tag="stat", name="r0")
        r1 = stat.tile([128, 1], F32, tag="stat", name="r1")
        rsum = stat.tile([128, 1], F32, tag="stat", name="rsum")
        nc.vector.tensor_reduce(r0[:], dist_t[0], axis=mybir.AxisListType.X,
                                op=ALU.add)
        nc.vector.tensor_reduce(r1[:], dist_t[1], axis=mybir.AxisListType.X,
                                op=ALU.add)
        nc.vector.tensor_add(rsum[:], r0[:], r1[:])
        ro0 = stat.tile([128, 1], F32, tag="stat", name="ro0")
        ro1 = stat.tile([128, 1], F32, tag="stat", name="ro1")
        rosum = stat.tile([128, 1], F32, tag="stat", name="rosum")
        nc.vector.tensor_reduce(ro0[:], disto_t[0][:],
                                axis=mybir.AxisListType.X, op=ALU.add)
        nc.vector.tensor_reduce(ro1[:], disto_t[1][:],
                                axis=mybir.AxisListType.X, op=ALU.add)
        nc.vector.tensor_add(rosum[:], ro0[:], ro1[:])
        tots = sb.tile([1, 2], F32, tag="tots", bufs=2, name="tots")
        tps_a = ps.tile([128, S], F32, tag="mm", name="tps_a")
        nc.tensor.matmul(tps_a[0:1, 0:1], ones[:, 0:1], rsum[:],
                         start=True, stop=True)
        nc.vector.tensor_copy(tots[:, 0:1], tps_a[0:1, 0:1])
        tps_b = ps.tile([128, S], F32, tag="mm", name="tps_b")
        nc.tensor.matmul(tps_b[0:1, 0:1], ones[:, 0:1], rosum[:],
                         start=True, stop=True)
        nc.vector.tensor_copy(tots[:, 1:2], tps_b[0:1, 0:1])

        arin3 = dram.tile([1, 2], F32, tag="arin3", name="arin3")
        arout3 = dram.tile([1, 2], F32, tag="arout3", name="arout3")
        nc.sync.dma_start(arin3[:], tots[:])
        nc.gpsimd.collective_compute(
            "AllReduce", ALU.add, replica_groups=[list(range(N_CORES))],
            ins=[arin3.opt()], outs=[arout3.opt()])
        ar3 = sb.tile([1, 2], F32, tag="tots", bufs=2, name="ar3")
        nc.sync.dma_start(ar3[:], arout3[:])
        means = stat.tile([1, 2], F32, tag="sqo", bufs=2, name="means")
        nc.vector.tensor_mul(means[:], ar3[:], scal[:])
        mb_ps = ps.tile([128, S], F32, tag="mm", name="mb_ps")
        nc.tensor.matmul(mb_ps[:, 0:2], ones[0:1, :], means[:],
                         start=True, stop=True)
        mean_bc = sb.tile([128, 2], F32, tag="meanbc", name="mean_bc")
        nc.vector.tensor_copy(mean_bc[:], mb_ps[:, 0:2])

        sigblk = sb.tile([128, NSC * H], F32, tag="mmout", name="sigblk")
        for qc in range(2):
            sig = sigblk[:, 0:S] if qc == 0 else sigblk[:, 3 * S:4 * S]
            nc.scalar.activation(sig, dist_t[qc], AF.Sigmoid,
                                 bias=mean_bc[:, 0:1], scale=-1.0)
            msk = sigblk[:, S:2 * S] if qc == 0 else sigblk[:, 4 * S:5 * S]
            nc.vector.tensor_scalar(msk, sig, THRESH, None, op0=ALU.is_ge)
            ww = sigblk[:, 2 * S:3 * S] if qc == 0 else sigblk[:, 5 * S:6 * S]
            nc.vector.tensor_mul(ww, sig, msk)
            nc.sync.dma_start(ww_out[qc * 128:(qc + 1) * 128, :], ww)

            sigo = sb.tile([128, NOPS], F32, tag="disto", bufs=6, name="sigo")
            nc.scalar.activation(sigo[:], disto_t[qc][:], AF.Sigmoid,
                                 bias=mean_bc[:, 1:2], scale=-1.0)
            msko = sb.tile([128, NOPS], F32, tag="disto", bufs=6, name="msko")
            nc.vector.tensor_scalar(msko[:], sigo[:], THRESH, None,
                                    op0=ALU.is_ge)
            wwo = sb.tile([128, NOPS], F32, tag="disto", bufs=6, name="wwo")
            nc.vector.tensor_mul(wwo[:], sigo[:], msko[:])
            nc.sync.dma_start(wop_out[qc * 128:(qc + 1) * 128, :], wwo[:])

        # embedded output (our half rows, chunk = 2*g + qc, blended by hsel)
        for qc in range(2):
            t0e = sb.tile([128, H], F32, tag="lns", bufs=4, name="t0e")
            nc.vector.tensor_scalar_mul(t0e[:], x[:, qc * H:(qc + 1) * H],
                                        hsel[:, 0:1])
            emb_sel = sb.tile([128, H], F32, tag="lns", bufs=4, name="emb_sel")
            nc.vector.scalar_tensor_tensor(
                emb_sel[:], x[:, (2 + qc) * H:(3 + qc) * H], hsel[:, 1:2],
                t0e[:], op0=ALU.mult, op1=ALU.add)
            nc.sync.dma_start(emb_out[qc * 128:(qc + 1) * 128, :], emb_sel[:])

    nc.finalize()
    return nc


# ---------------------------------------------------------------------------
# Host side: input prep, runner, output assembly
# ---------------------------------------------------------------------------

_RUNNER = None


class _Runner:
    def __init__(self):
        self.nc = build_program()
        nc = self.nc
        bass2jax.install_neuronx_cc_hook()
        partition_name = (nc.partition_id_tensor.name
                          if nc.partition_id_tensor else None)
        in_names, out_names, out_avals, zero_outs = [], [], [], []
        for alloc in nc.m.functions[0].allocations:
            if not isinstance(alloc, mybir.MemoryLocationSet):
                continue
            name = alloc.memorylocations[0].name
            if alloc.kind == "ExternalInput":
                if name != partition_name:
                    in_names.append(name)
            elif alloc.kind == "ExternalOutput":
                out_names.append(name)
                shape = tuple(alloc.tensor_shape)
                dtype = mybir.dt.np(alloc.dtype)
                out_avals.append(jax.core.ShapedArray(shape, dtype))
                zero_outs.append(np.zeros(shape, dtype))
        self.n_params = len(in_names)
        self.param_names = list(in_names)
        self.out_names = out_names
        self.out_avals = out_avals
        self.zero_outs = zero_outs
        all_in = in_names + out_names
        if partition_name:
            all_in.append(partition_name)

        def _body(*args):
            operands = list(args)
            if partition_name:
                operands.append(bass2jax.partition_id_tensor())
            outs = bass2jax._bass_exec_p.bind(
                *operands, out_avals=tuple(out_avals),
                in_names=tuple(all_in), out_names=tuple(out_names),
                lowering_input_output_aliases=(), sim_require_finite=True,
                sim_require_nnan=True, nc=nc)
            return tuple(outs)

        devices = jax.devices()[:N_CORES]
        self.mesh = Mesh(np.asarray(devices), ("core",))
        n_in = self.n_params + len(out_names)
        self.sharded = jax.jit(
            shard_map(_body, mesh=self.mesh,
                      in_specs=(PartitionSpec("core"),) * n_in,
                      out_specs=(PartitionSpec("core"),) * len(out_names),
                      check_rep=False),
            keep_unused=True)
        self.sharding = NamedSharding(self.mesh, PartitionSpec("core"))

    def place(self, per_core_maps):
        args = []
        for name in self.param_names:
            cat = np.concatenate([np.asarray(m[name]) for m in per_core_maps],
                                 axis=0)
            args.append(jax.device_put(cat, self.sharding))
        for z in self.zero_outs:
            cat = np.zeros((N_CORES * z.shape[0], *z.shape[1:]), z.dtype)
            args.append(jax.device_put(cat, self.sharding))
        return args

    def run_raw(self, args):
        return self.sharded(*args)

    def run(self, args):
        outs = self.sharded(*args)
        jax.block_until_ready(outs)
        return {name: np.asarray(o).reshape(N_CORES, *self.out_avals[i].shape)
                for i, (name, o) in enumerate(
                    zip(self.out_names, outs))}


def get_runner():
    global _RUNNER
    if _RUNNER is None:
        _RUNNER = _Runner()
    return _RUNNER


def prepare_core_inputs(inputs, core):
    p, g = core // 2, core % 2
    f32 = np.float32
    ids = np.asarray(inputs["input_ids"])[p]
    x0 = (np.asarray(inputs["word_emb"], f32)[ids]
          + np.asarray(inputs["pos_emb"], f32)[:S]
          + np.asarray(inputs["type_emb"], f32)[0][None, :]).astype(f32)
    am = np.asarray(inputs["attention_mask"])[p].astype(f32)
    amask = np.ascontiguousarray(
        ((1.0 - am) * -1e9).astype(f32).reshape(NSC, 128).T)

    rep = lambda v: np.repeat(np.asarray(v, f32).reshape(1, -1), 128, axis=0)
    embg = rep(inputs["emb_ln_g"])
    embb = rep(inputs["emb_ln_b"])

    Wqkv = np.asarray(inputs["Wqkv"], f32)
    bqkv = np.asarray(inputs["bqkv"], f32)
    Wo = np.asarray(inputs["Wo"], f32)
    bo = np.asarray(inputs["bo"], f32)
    W1 = np.asarray(inputs["W1"], f32)
    b1 = np.asarray(inputs["b1"], f32)
    W2 = np.asarray(inputs["W2"], f32)
    b2 = np.asarray(inputs["b2"], f32)

    qs = slice(g * HHALF, (g + 1) * HHALF)
    ks = slice(H + g * HHALF, H + (g + 1) * HHALF)
    vs = slice(2 * H + g * HHALF, 2 * H + (g + 1) * HHALF)
    wqkv = np.ascontiguousarray(
        np.concatenate([Wqkv[:, :, qs], Wqkv[:, :, ks], Wqkv[:, :, vs]],
                       axis=2))[:NL]
    wo = np.ascontiguousarray(Wo[:, g * HHALF:(g + 1) * HHALF, :])[:NL]
    w1 = np.ascontiguousarray(W1[:, :, g * FFHALF:(g + 1) * FFHALF])[:NL]
    w2 = np.ascontiguousarray(W2[:, g * FFHALF:(g + 1) * FFHALF, :])[:NL]

    nl = NL
    qkb = np.zeros((nl, 128, 2 * NQK + NFFC), f32)
    for m in range(NQK):
        qkb[:, :, m] = bqkv[:nl, g * HHALF + m * 128: g * HHALF + (m + 1) * 128]
        qkb[:, :, NQK + m] = bqkv[:nl, H + g * HHALF + m * 128:
                                  H + g * HHALF + (m + 1) * 128]
    for m in range(NFFC):
        qkb[:, :, 2 * NQK + m] = b1[:nl, g * FFHALF + m * 128:
                                    g * FFHALF + (m + 1) * 128]
    vb = np.repeat(bqkv[:nl, None, 2 * H + g * HHALF: 2 * H + (g + 1) * HHALF],
                   128, axis=1).astype(f32)
    repl = lambda a: np.repeat(np.asarray(a, f32)[:nl, None, :], 128, axis=1)
    ob = repl(bo)
    b2r = repl(b2)
    ln1g = repl(inputs["ln1_g"])
    ln1b = repl(inputs["ln1_b"])
    ln2g = repl(inputs["ln2_g"])
    ln2b = repl(inputs["ln2_b"])

    opT = np.ascontiguousarray(np.asarray(inputs["op_emb"], f32).T)
    scal = np.array([[1.0 / (B * S * S), 1.0 / (B * S * NOPS)]], f32)
    hsel = np.zeros((128, 2), f32)
    hsel[:, g] = 1.0

    return {
        "x0": x0, "amask": amask, "embg": embg, "embb": embb,
        "wqkv": wqkv, "wo": wo, "w1": w1, "w2": w2,
        "qkb": qkb, "vb": vb, "ob": ob, "b2": b2r,
        "ln1g": ln1g, "ln1b": ln1b, "ln2g": ln2g, "ln2b": ln2b,
        "opT": opT, "scal": scal, "hsel": hsel,
    }


def assemble_outputs(res):
    f32 = np.float32
    embedded = np.zeros((B, S, H), f32)
    word_word = np.zeros((B, S, S), f32)
    word_operator = np.zeros((B, S, NOPS), f32)
    for c in range(N_CORES):
        p, g = c // 2, c % 2
        rows = slice(g * SQ, (g + 1) * SQ)
        embedded[p, rows] = res["emb_out"][c]
        word_word[p, rows] = res["ww_out"][c]
        word_operator[p, rows] = res["wop_out"][c]
    return embedded, word_word, word_operator


def kernel(**inputs):
    r = get_runner()
    maps = [prepare_core_inputs(inputs, c) for c in range(N_CORES)]
    args = r.place(maps)
    res = r.run(args)
    return assemble_outputs(res)
